# revision 1
# baseline (speedup 1.0000x reference)
"""Trainium2 Bass kernel for the BayesianSkipgram loss.

Strategy (8 NeuronCores, batch-sharded, no collectives):
  - Each core computes the per-sample loss for its 256-sample batch shard.
  - Encoder runs in "T layout" ([D partitions, sample free-dim]) so every
    matmul contraction lands on the partition axis with host-pretransposed
    weights (no on-device weight transposes).
  - The dominant [B, N] logits are never materialized: for each batch chunk
    of 128, logits stream through PSUM in 2048-column groups (4 banks), and
    ScalarE computes Exp with a fused per-partition accumulate (accum_out)
    giving sum(exp(logits)) directly. W_gen^T is uploaded pre-cast to bf16
    and held resident in SBUF (~98KB/partition).
  - take_along(logp) is computed exactly as z . W_gen[ctx] + b_gen[ctx]
    - C*logsumexp, via fp32 indirect-DMA row gathers (one per context
    position) dotted against z in natural layout. b_gen rides along as a
    129th column of W_gen so the bias is part of the same dot product.
  - Host combines the 8x[256] per-sample losses with a mean.
"""

import numpy as np
import ml_dtypes

import concourse.bass as bass
import concourse.mybir as mybir
import concourse.tile as tile
from concourse import bacc
from concourse.bass import IndirectOffsetOnAxis
from concourse.bass_utils import run_bass_kernel_spmd

F32 = mybir.dt.float32
BF16 = mybir.dt.bfloat16
I32 = mybir.dt.int32
AF = mybir.ActivationFunctionType
ALU = mybir.AluOpType

N = 50257      # vocab
D = 128        # embedding dim
B = 2048       # total batch
C = 10         # context size
NCORES = 8
BS = B // NCORES          # 256 samples per core
M = BS // 128             # 2 partition chunks of samples
NFLAT = BS * C            # 2560 flattened (sample, ctx) pairs per core
NT = NFLAT // 128         # 20 gather tiles
GROUP = 2048              # psum group (4 banks of fp32)
NGROUPS = (N + GROUP - 1) // GROUP   # 25
CHUNK = 512               # matmul free-dim (one psum bank)

_CACHE = {}


def _patch_act_tables():
    """Keep Exp/Ln/Identity/Copy only in natural_log_exp_and_others so the
    table-load inserter uses one set for the whole kernel (one ~2.7us load
    instead of thrashing between exp_and_others / natural_log)."""
    import concourse.bacc as _bacc_mod
    import concourse.hw_specs as _hws
    if getattr(_bacc_mod, "_ant_act_tables_patched", False):
        return
    _orig = _hws.get_activation_tables
    _ours = {AF.Exp, AF.Ln, AF.Identity, AF.Copy}

    def _filtered(arch):
        tabs = _orig(arch)
        out = {}
        for name, funcs in tabs.items():
            if name == "natural_log_exp_and_others" or not (_ours & funcs):
                out[name] = funcs
            else:
                out[name] = funcs - _ours
        return out

    _bacc_mod.get_activation_tables = _filtered
    _bacc_mod._ant_act_tables_patched = True


def _build(with_bgen: bool, stage: int = 99):
    """stage: dev-only truncation knob. 1=io, 2=+gathers, 3=+encoder/kl/
    takealong, 4=+bigloop, 99=full."""
    _patch_act_tables()
    nc = bacc.Bacc("TRN2", target_bir_lowering=False, debug=False)

    # ---------------- DRAM I/O ----------------
    d_ctx_idx = nc.dram_tensor("ctx_idx", [128, NT], I32, kind="ExternalInput")
    d_ctx_byc = nc.dram_tensor("ctx_byc", [128, M * C], I32, kind="ExternalInput")
    d_x_idx = nc.dram_tensor("x_idx", [128, M], I32, kind="ExternalInput")
    d_epsT = nc.dram_tensor("epsT", [128, BS], F32, kind="ExternalInput")
    d_waff1T = nc.dram_tensor("waff1T", [128, 128], F32, kind="ExternalInput")
    d_waff2T = nc.dram_tensor("waff2T", [128, 128], F32, kind="ExternalInput")
    d_wmuT = nc.dram_tensor("wmuT", [128, 128], F32, kind="ExternalInput")
    d_wsigT = nc.dram_tensor("wsigT", [128, 128], F32, kind="ExternalInput")
    d_baff = nc.dram_tensor("baff", [128, 1], F32, kind="ExternalInput")
    d_bmu = nc.dram_tensor("bmu", [128, 1], F32, kind="ExternalInput")
    d_bsig = nc.dram_tensor("bsig", [128, 1], F32, kind="ExternalInput")
    d_inf_emb = nc.dram_tensor("inf_emb", [N, D], F32, kind="ExternalInput")
    d_gsig_emb = nc.dram_tensor("gsig_emb", [N, D], F32, kind="ExternalInput")
    d_wg_aug = nc.dram_tensor("wg_aug", [N, D + 1], F32, kind="ExternalInput")
    d_wgT = nc.dram_tensor("wgT", [128, N], BF16, kind="ExternalInput")
    d_ident = nc.dram_tensor("ident", [128, 128], F32, kind="ExternalInput")
    if with_bgen:
        NCH = (N + CHUNK - 1) // CHUNK  # 99
        d_bgen = nc.dram_tensor("bgen2d", [NCH, CHUNK], BF16, kind="ExternalInput")
    d_loss = nc.dram_tensor("loss_part", [BS], F32, kind="ExternalOutput")

    with tile.TileContext(nc) as tc:
        cpool = tc.alloc_tile_pool(name="consts", bufs=1)
        wgpool = tc.alloc_tile_pool(name="wg", bufs=1)
        encpool = tc.alloc_tile_pool(name="enc", bufs=1)
        gpool = tc.alloc_tile_pool(name="gath", bufs=3)
        dpool = tc.alloc_tile_pool(name="dram", bufs=1, space="DRAM")

        # ---- constants / small inputs ----
        # identity comes from the host: make_identity would occupy GpSimd for
        # ~7us and delay the serial indirect-gather chain behind it
        identity = cpool.tile([128, 128], F32)
        nc.sync.dma_start(out=identity[:], in_=d_ident[:, :])
        ones_col = cpool.tile([128, 1], F32)
        nc.vector.memset(ones_col, 1.0)

        ctxi = cpool.tile([128, NT], I32)
        nc.sync.dma_start(out=ctxi[:], in_=d_ctx_idx[:, :])
        ctxbc = cpool.tile([128, M * C], I32)
        nc.sync.dma_start(out=ctxbc[:], in_=d_ctx_byc[:, :])
        xi = cpool.tile([128, M], I32)
        nc.sync.dma_start(out=xi[:], in_=d_x_idx[:, :])

        epsT = cpool.tile([128, BS], F32)
        nc.sync.dma_start(out=epsT[:], in_=d_epsT[:, :])
        waff1T = cpool.tile([128, 128], F32)
        nc.sync.dma_start(out=waff1T[:], in_=d_waff1T[:, :])
        waff2T = cpool.tile([128, 128], F32)
        nc.sync.dma_start(out=waff2T[:], in_=d_waff2T[:, :])
        wmuT = cpool.tile([128, 128], F32)
        nc.sync.dma_start(out=wmuT[:], in_=d_wmuT[:, :])
        wsigT = cpool.tile([128, 128], F32)
        nc.sync.dma_start(out=wsigT[:], in_=d_wsigT[:, :])
        baff = cpool.tile([128, 1], F32)
        nc.sync.dma_start(out=baff[:], in_=d_baff[:, :])
        bmu = cpool.tile([128, 1], F32)
        nc.sync.dma_start(out=bmu[:], in_=d_bmu[:, :])
        bsig = cpool.tile([128, 1], F32)
        nc.sync.dma_start(out=bsig[:], in_=d_bsig[:, :])

        # ---- resident W_gen^T (bf16), loaded in column blocks (emitted after
        # the small input DMAs so they don't queue behind 13MB of weights) ----
        wg_sb = wgpool.tile([128, N], BF16)
        if stage >= 4:
            WBLK = 4096
            for c0 in range(0, N, WBLK):
                c1 = min(c0 + WBLK, N)
                nc.sync.dma_start(out=wg_sb[:, c0:c1], in_=d_wgT[:, c0:c1])

        # persistent encoder tensors
        ctxT = encpool.tile([128, NFLAT], F32)
        centerT = encpool.tile([128, BS], F32)
        gsigT = encpool.tile([128, BS], F32)
        h_pre = encpool.tile([128, NFLAT], F32)
        h_sumT = encpool.tile([128, BS], F32)
        muT = encpool.tile([128, BS], F32)
        infsigT = encpool.tile([128, BS], F32)
        sigmaT = encpool.tile([128, BS], F32)
        zT = encpool.tile([128, BS], F32)
        z_bf = encpool.tile([128, BS], BF16)
        z_nat = encpool.tile([128, M * (D + 1)], F32)   # [z | 1] natural per m
        tal_bc = encpool.tile([128, M * C], F32)
        sums = encpool.tile([128, M * 32], F32)     # exp partial sums, col g/group
        talsum = encpool.tile([128, M], F32)
        lse = encpool.tile([128, M], F32)
        kl_rt = encpool.tile([128, M], F32)
        tot = encpool.tile([128, M], F32)
        loss_sb = encpool.tile([128, M], F32)

        kl_d = dpool.tile([BS], F32)

        if stage < 99:
            nc.vector.memset(loss_sb[:, :], float(stage))

        with tc.tile_pool(name="encps", bufs=2, space="PSUM") as encps, \
             tc.tile_pool(name="mmps", bufs=1, space="PSUM") as mmps:

            # ---- gathers + PE transposes ----
            # (one indirect DMA per 128 indices: multi-index offset APs pass
            # CoreSim but return garbage on hardware SWDGE)
            def gather_multi(src_dram, idx_ap, nat_tile, k):
                d = src_dram.shape[1]
                for t in range(k):
                    nc.gpsimd.indirect_dma_start(
                        out=nat_tile[:, t * d:(t + 1) * d],
                        out_offset=None,
                        in_=src_dram[:, :],
                        in_offset=IndirectOffsetOnAxis(
                            ap=idx_ap[:, t:t + 1], axis=0),
                    )

            def transpose_to(nat_tile, dstT, k):
                for t in range(k):
                    ps = encps.tile([128, 128], F32, tag="tps")
                    nc.tensor.transpose(out=ps[:, :],
                                        in_=nat_tile[:, t * 128:(t + 1) * 128],
                                        identity=identity[:, :])
                    nc.vector.tensor_copy(out=dstT[:, t * 128:(t + 1) * 128],
                                          in_=ps[:, :])

            if stage >= 2:
                ctx_nat = encpool.tile([128, NFLAT], F32)
                gather_multi(d_inf_emb, ctxi[:, :], ctx_nat, NT)
                cg_nat = encpool.tile([128, 2 * M * 128], F32)
                gather_multi(d_inf_emb, xi[:, :], cg_nat[:, :M * 128], M)
                gather_multi(d_gsig_emb, xi[:, :], cg_nat[:, M * 128:], M)
                transpose_to(ctx_nat, ctxT, NT)
                transpose_to(cg_nat[:, :M * 128], centerT, M)
                transpose_to(cg_nat[:, M * 128:], gsigT, M)

            if stage >= 3:
                # ---- encoder ----
                apre_ps = mmps.tile([128, BS], F32, tag="apre")
                nc.tensor.matmul(out=apre_ps[:, :], lhsT=waff1T[:, :],
                                 rhs=centerT[:, :], start=True, stop=True)
                apre = encpool.tile([128, BS], F32)
                nc.scalar.activation(out=apre[:, :], in_=apre_ps[:, :],
                                     func=AF.Identity, bias=baff[:, 0:1], scale=1.0)

                # Bpre in two b-aligned halves (1280 cols = 128 samples * C)
                HALF = NFLAT // 2
                for hh in range(2):
                    bpre_ps = mmps.tile([128, HALF], F32, tag="bpre")
                    r0 = hh * HALF
                    for j0 in range(0, HALF, CHUNK):
                        j1 = min(j0 + CHUNK, HALF)
                        nc.tensor.matmul(out=bpre_ps[:, j0:j1],
                                         lhsT=waff2T[:, :],
                                         rhs=ctxT[:, r0 + j0:r0 + j1],
                                         start=True, stop=True)
                    bpre3 = bpre_ps[:, :].rearrange("p (b c) -> p b c", c=C)
                    h3 = h_pre[:, r0:r0 + HALF].rearrange("p (b c) -> p b c", c=C)
                    nc.vector.tensor_tensor(
                        out=h3, in0=bpre3,
                        in1=apre[:, hh * 128:(hh + 1) * 128]
                        .to_broadcast([128, 128, C]),
                        op=ALU.add)
                nc.vector.tensor_scalar_max(out=h_pre[:, :], in0=h_pre[:, :],
                                            scalar1=0.0)
                nc.vector.reduce_sum(
                    out=h_sumT[:, :],
                    in_=h_pre[:, :].rearrange("p (b c) -> p b c", c=C),
                    axis=mybir.AxisListType.X)

                # mu / inf_sigma
                mu_ps = mmps.tile([128, BS], F32, tag="apre")
                nc.tensor.matmul(out=mu_ps[:, :], lhsT=wmuT[:, :], rhs=h_sumT[:, :],
                                 start=True, stop=True)
                nc.scalar.activation(out=muT[:, :], in_=mu_ps[:, :],
                                     func=AF.Identity, bias=bmu[:, 0:1], scale=1.0)

                sig_ps = mmps.tile([128, BS], F32, tag="apre")
                nc.tensor.matmul(out=sig_ps[:, :], lhsT=wsigT[:, :],
                                 rhs=h_sumT[:, :], start=True, stop=True)
                # softplus(x) = ln(1 + exp(x)) via Exp/Ln (one ACT table set)
                sp_e = encpool.tile([128, BS], F32)
                nc.scalar.activation(out=sp_e[:, :], in_=sig_ps[:, :],
                                     func=AF.Exp, bias=bsig[:, 0:1], scale=1.0)
                nc.vector.tensor_scalar_add(out=sp_e[:, :], in0=sp_e[:, :],
                                            scalar1=1.0)
                nc.scalar.activation(out=infsigT[:, :], in_=sp_e[:, :], func=AF.Ln)

                sp_g = encpool.tile([128, BS], F32)
                nc.scalar.activation(out=sp_g[:, :], in_=gsigT[:, :], func=AF.Exp)
                nc.vector.tensor_scalar_add(out=sp_g[:, :], in0=sp_g[:, :],
                                            scalar1=1.0)
                nc.scalar.activation(out=sigmaT[:, :], in_=sp_g[:, :], func=AF.Ln)

                # z = mu + eps * inf_sigma
                nc.vector.tensor_tensor(out=zT[:, :], in0=epsT[:, :],
                                        in1=infsigT[:, :], op=ALU.mult)
                nc.vector.tensor_tensor(out=zT[:, :], in0=zT[:, :], in1=muT[:, :],
                                        op=ALU.add)
                nc.vector.tensor_copy(out=z_bf[:, :], in_=zT[:, :])

                # ---- KL per-sample (partition-reduce via ones matmul) ----
                kli = encpool.tile([128, BS], F32)
                tmp = encpool.tile([128, BS], F32)
                nc.scalar.activation(out=kli[:, :], in_=sigmaT[:, :], func=AF.Ln)
                nc.scalar.activation(out=tmp[:, :], in_=infsigT[:, :], func=AF.Ln)
                nc.vector.tensor_tensor(out=kli[:, :], in0=kli[:, :], in1=tmp[:, :],
                                        op=ALU.subtract)
                num = encpool.tile([128, BS], F32)
                nc.vector.tensor_tensor(out=num[:, :], in0=muT[:, :],
                                        in1=sigmaT[:, :], op=ALU.subtract)
                nc.vector.tensor_tensor(out=num[:, :], in0=num[:, :], in1=num[:, :],
                                        op=ALU.mult)
                nc.vector.tensor_tensor(out=tmp[:, :], in0=infsigT[:, :],
                                        in1=infsigT[:, :], op=ALU.mult)
                nc.vector.tensor_tensor(out=num[:, :], in0=num[:, :], in1=tmp[:, :],
                                        op=ALU.add)
                nc.vector.tensor_tensor(out=tmp[:, :], in0=sigmaT[:, :],
                                        in1=sigmaT[:, :], op=ALU.mult)
                nc.vector.tensor_scalar_mul(out=tmp[:, :], in0=tmp[:, :],
                                            scalar1=2.0)
                nc.vector.reciprocal(out=tmp[:, :], in_=tmp[:, :])
                nc.vector.tensor_tensor(out=num[:, :], in0=num[:, :], in1=tmp[:, :],
                                        op=ALU.mult)
                nc.vector.tensor_tensor(out=kli[:, :], in0=kli[:, :], in1=num[:, :],
                                        op=ALU.add)
                nc.vector.tensor_scalar_add(out=kli[:, :], in0=kli[:, :],
                                            scalar1=-0.5)
                kl_ps = mmps.tile([1, BS], F32, tag="klps")
                nc.tensor.matmul(out=kl_ps[:, :], lhsT=ones_col[:, :],
                                 rhs=kli[:, :], start=True, stop=True)
                kl_row = encpool.tile([1, BS], F32)
                nc.vector.tensor_copy(out=kl_row[:, :], in_=kl_ps[:, :])
                nc.sync.dma_start(out=kl_d[:].rearrange("(a b) -> a b", a=1),
                                  in_=kl_row[:, :])
                nc.sync.dma_start(out=kl_rt[:, :],
                                  in_=kl_d[:].rearrange("(m p) -> p m", p=128))

                # ---- z in natural layout, augmented with a ones column ----
                for m in range(M):
                    zps = encps.tile([128, 128], F32, tag="tps")
                    nc.tensor.transpose(out=zps[:, :],
                                        in_=zT[:, m * 128:(m + 1) * 128],
                                        identity=identity[:, :])
                    a0 = m * (D + 1)
                    nc.vector.tensor_copy(out=z_nat[:, a0:a0 + D], in_=zps[:, :])
                    nc.vector.tensor_copy(out=z_nat[:, a0 + D:a0 + D + 1],
                                          in_=ones_col[:, :])

                # ---- take-along dots: tal_bc[p, m*C+c] =
                #      z_aug[m*128+p] . wg_aug[ctx[m*128+p, c]] ----
                dummy = encpool.tile([128, D + 1], F32)
                wrows = encpool.tile([128, M * C * (D + 1)], F32)
                gather_multi(d_wg_aug, ctxbc[:, :], wrows, M * C)
                for m in range(M):
                    a0 = m * (D + 1)
                    for c in range(C):
                        col = m * C + c
                        w0 = col * (D + 1)
                        nc.vector.tensor_tensor(
                            out=dummy[:, :], in0=wrows[:, w0:w0 + D + 1],
                            in1=z_nat[:, a0:a0 + D + 1], op=ALU.mult)
                        nc.vector.reduce_sum(out=tal_bc[:, col:col + 1],
                                             in_=dummy[:, :],
                                             axis=mybir.AxisListType.X)
                for m in range(M):
                    nc.vector.reduce_sum(out=talsum[:, m:m + 1],
                                         in_=tal_bc[:, m * C:(m + 1) * C],
                                         axis=mybir.AxisListType.X)

        # ---- the big streamed logits/exp loop ----
        if with_bgen and stage >= 4:
            ones_bf = cpool.tile([1, 128], BF16)
            nc.vector.memset(ones_bf, 1.0)
        with tc.tile_pool(name="bigps", bufs=2, space="PSUM") as bigps, \
             tc.tile_pool(name="expo", bufs=2) as expop, \
             tc.tile_pool(name="bgst", bufs=4) as bgstp:
            if stage >= 4:
                for m in range(M):
                    zcol = z_bf[:, m * 128:(m + 1) * 128]
                    for g in range(NGROUPS):
                        c0 = g * GROUP
                        gw = min(GROUP, N - c0)
                        ps = bigps.tile([128, GROUP], F32, tag="big")
                        for j0 in range(0, gw, CHUNK):
                            j1 = min(j0 + CHUNK, gw)
                            nc.tensor.matmul(
                                out=ps[:, j0:j1], lhsT=zcol,
                                rhs=wg_sb[:, c0 + j0:c0 + j1],
                                start=True, stop=not with_bgen,
                            )
                            if with_bgen:
                                bg = bgstp.tile([1, CHUNK], BF16, tag="bg")
                                nc.sync.dma_start(
                                    out=bg[:, :j1 - j0],
                                    in_=d_bgen[(c0 + j0) // CHUNK, :j1 - j0]
                                    .rearrange("(a b) -> a b", a=1))
                                nc.tensor.matmul(
                                    out=ps[:, j0:j1], lhsT=ones_bf[:, :],
                                    rhs=bg[:, :j1 - j0], start=False, stop=True,
                                )
                        eo = expop.tile([128, GROUP], BF16, tag="eo")
                        nc.scalar.activation(
                            out=eo[:, :gw], in_=ps[:, :gw], func=AF.Exp,
                            accum_out=sums[:, m * 32 + g:m * 32 + g + 1],
                        )

        # ---- epilogue: lse, loss assembly ----
        if stage >= 99:
            for m in range(M):
                nc.vector.reduce_sum(out=tot[:, m:m + 1],
                                     in_=sums[:, m * 32:m * 32 + NGROUPS],
                                     axis=mybir.AxisListType.X)
            nc.scalar.activation(out=lse[:, :], in_=tot[:, :], func=AF.Ln)
            # loss = kl - talong_sum + C * lse
            nc.vector.tensor_tensor(out=loss_sb[:, :], in0=kl_rt[:, :],
                                    in1=talsum[:, :], op=ALU.subtract)
            nc.vector.tensor_scalar_mul(out=lse[:, :], in0=lse[:, :],
                                        scalar1=float(C))
            nc.vector.tensor_tensor(out=loss_sb[:, :], in0=loss_sb[:, :],
                                    in1=lse[:, :], op=ALU.add)
        nc.sync.dma_start(out=d_loss[:].rearrange("(m p) -> p m", p=128),
                          in_=loss_sb[:, :])

        dpool.release()
        gpool.release()
        encpool.release()
        wgpool.release()
        cpool.release()

    nc.compile()
    return nc


def _prep_inputs(x_batch, context_words_batch, eps, inf_emb, W_aff, b_aff,
                 W_mu, b_mu, W_sig, b_sig, gen_sigma_emb, W_gen, b_gen,
                 with_bgen):
    f32 = lambda a: np.ascontiguousarray(np.asarray(a, dtype=np.float32))
    x_batch = np.asarray(x_batch, dtype=np.int32)
    ctx = np.asarray(context_words_batch, dtype=np.int32)
    eps = f32(eps)
    W_aff, W_mu, W_sig = f32(W_aff), f32(W_mu), f32(W_sig)
    b_aff, b_mu, b_sig = f32(b_aff), f32(b_mu), f32(b_sig)
    inf_emb, gen_sigma_emb = f32(inf_emb), f32(gen_sigma_emb)
    W_gen, b_gen = f32(W_gen), f32(b_gen)

    shared = {
        "waff1T": np.ascontiguousarray(W_aff[:, :D].T),
        "waff2T": np.ascontiguousarray(W_aff[:, D:].T),
        "wmuT": np.ascontiguousarray(W_mu.T),
        "wsigT": np.ascontiguousarray(W_sig.T),
        "baff": np.ascontiguousarray(b_aff.reshape(D, 1)),
        "bmu": np.ascontiguousarray(b_mu.reshape(D, 1)),
        "bsig": np.ascontiguousarray(b_sig.reshape(D, 1)),
        "inf_emb": inf_emb,
        "gsig_emb": gen_sigma_emb,
        "wg_aug": np.ascontiguousarray(
            np.concatenate([W_gen, b_gen.reshape(N, 1)], axis=1)),
        "wgT": np.ascontiguousarray(W_gen.T.astype(ml_dtypes.bfloat16)),
        "ident": np.eye(128, dtype=np.float32),
    }
    if with_bgen:
        NCH = (N + CHUNK - 1) // CHUNK
        bg = np.zeros((NCH * CHUNK,), dtype=ml_dtypes.bfloat16)
        bg[:N] = b_gen.astype(ml_dtypes.bfloat16)
        shared["bgen2d"] = bg.reshape(NCH, CHUNK)

    in_maps = []
    for s in range(NCORES):
        lo, hi = s * BS, (s + 1) * BS
        csh = ctx[lo:hi]                      # [BS, C]
        m = dict(shared)
        m["ctx_idx"] = np.ascontiguousarray(
            csh.reshape(NFLAT).reshape(NT, 128).T)
        m["ctx_byc"] = np.ascontiguousarray(
            np.concatenate([csh[k * 128:(k + 1) * 128, :] for k in range(M)],
                           axis=1))
        m["x_idx"] = np.ascontiguousarray(x_batch[lo:hi].reshape(M, 128).T)
        m["epsT"] = np.ascontiguousarray(eps[lo:hi].T)
        in_maps.append(m)
    return in_maps


def kernel(x_batch, context_words_batch, eps, inf_emb, W_aff, b_aff,
           W_mu, b_mu, W_sig, b_sig, gen_sigma_emb, W_gen, b_gen,
           trace=False):
    with_bgen = bool(np.any(np.asarray(b_gen) != 0))
    if with_bgen not in _CACHE:
        _CACHE[with_bgen] = _build(with_bgen)
    nc = _CACHE[with_bgen]

    in_maps = _prep_inputs(x_batch, context_words_batch, eps, inf_emb, W_aff,
                           b_aff, W_mu, b_mu, W_sig, b_sig, gen_sigma_emb,
                           W_gen, b_gen, with_bgen)
    res = run_bass_kernel_spmd(nc, in_maps, core_ids=list(range(NCORES)),
                               trace=trace)
    parts = [res.results[s]["loss_part"] for s in range(NCORES)]
    loss = np.concatenate(parts).astype(np.float64).mean()
    out = np.float32(loss)
    if trace:
        kernel.last_results = res
    return out



# revision 3
# speedup vs baseline: 1.0084x; 1.0084x over previous
"""Trainium2 Bass kernel for the BayesianSkipgram loss.

Strategy (8 NeuronCores, batch-sharded, no collectives):
  - Each core computes the per-sample loss for its 256-sample batch shard.
  - The dominant cost is sum(exp(logits)) over the 50257-vocab: the ACT
    engine's Exp throughput (0.833ns/elem) is the roofline. Everything else
    is scheduled to hide under the two 33-group EXP trains (one per
    128-sample chunk m).
  - Pipelined start: chunk m0's gathers + encoder run first so the m0 EXP
    train starts at ~24us; m1's gathers/encoder/KL/take-along all overlap
    the m0 train via careful per-engine emission interleaving.
  - W_gen^T is pre-scaled x8 and cast to fp8e4 on host (6.4MB), streamed in
    28 column-slices so group 0 lands before the first matmul. z is scaled
    1/8 to fp8 for the logits matmul (PSUM accumulates f32); logit rms
    error ~4% of logit sigma, irrelevant for logsumexp at 2e-2 tolerance.
  - PSUM: 6 banks = 2 x [128,1536] streaming tiles, 2 banks = encoder ring
    (transposes + small matmuls), so encoder m1 overlaps the m0 train.
  - take_along(logp) is exact: fp32 indirect-DMA row gathers of
    [W_gen | b_gen] dotted against fp32 z (tensor_tensor_reduce).
  - Optionally (SCHRAUD_PERIOD) a fraction of vocab groups compute
    sum-exp on the DVE via a Schraudolph bit-trick exp (int16 + bitcast
    bf16), offloading the ACT engine.
  - Host combines the 8x[256] per-sample losses with a mean.
"""

import numpy as np
import ml_dtypes

import concourse.bass as bass
import concourse.mybir as mybir
import concourse.tile as tile
from concourse import bacc
from concourse.bass import IndirectOffsetOnAxis
from concourse.bass_utils import run_bass_kernel_spmd

F32 = mybir.dt.float32
BF16 = mybir.dt.bfloat16
FP8 = mybir.dt.float8e4
I16 = mybir.dt.int16
I32 = mybir.dt.int32
AF = mybir.ActivationFunctionType
ALU = mybir.AluOpType
AXX = mybir.AxisListType.X

N = 50257      # vocab
D = 128        # embedding dim
B = 2048       # total batch
C = 10         # context size
NCORES = 8
BS = B // NCORES          # 256 samples per core
M = BS // 128             # 2 partition chunks of samples
NT = BS * C // 128        # 20 ctx gather tiles (10 per m)
TPM = NT // M             # 10
GROUP = 1536              # streaming group (3 psum banks)
NG = (N + GROUP - 1) // GROUP        # 33 groups
CHUNK = 512               # matmul free-dim
WSCALE = 8.0              # W_gen pre-scale (z scaled 1/WSCALE)

# Schraudolph DVE-exp offload: every SCHRAUD_PERIOD-th group (within an m)
# is computed on the DVE instead of ACT. 0 = disabled (all exact ACT exp).
SCHRAUD_PERIOD = 0
SCH_C1 = 184.6650390625   # 2^7 * log2(e)
SCH_C2 = 16250.25         # calibrated: 127*2^7 minus mean-centering tweak

# W load slicing: first fine slices for fast group-0 arrival, then coarse.
W_SLICES = [1024] * 6 + [2048] * 22   # 6*1024 + 22*2048 = 51200 >= N

_CACHE = {}


def _patch_act_tables():
    """Keep Exp/Ln/Identity/Copy only in natural_log_exp_and_others so the
    table-load inserter uses one set for the whole kernel."""
    import concourse.bacc as _bacc_mod
    import concourse.hw_specs as _hws
    if getattr(_bacc_mod, "_ant_act_tables_patched", False):
        return
    _orig = _hws.get_activation_tables
    _ours = {AF.Exp, AF.Ln, AF.Identity, AF.Copy}

    def _filtered(arch):
        tabs = _orig(arch)
        out = {}
        for name, funcs in tabs.items():
            if name == "natural_log_exp_and_others" or not (_ours & funcs):
                out[name] = funcs
            else:
                out[name] = funcs - _ours
        return out

    _bacc_mod.get_activation_tables = _filtered
    _bacc_mod._ant_act_tables_patched = True


def _build(with_bgen: bool):
    _patch_act_tables()
    nc = bacc.Bacc("TRN2", target_bir_lowering=False, debug=False)

    # ---------------- DRAM I/O ----------------
    d_ctx_idx = nc.dram_tensor("ctx_idx", [128, NT], I32, kind="ExternalInput")
    d_ctx_byc = nc.dram_tensor("ctx_byc", [128, M * C], I32, kind="ExternalInput")
    d_x_idx = nc.dram_tensor("x_idx", [128, M], I32, kind="ExternalInput")
    d_epsT = nc.dram_tensor("epsT", [128, BS], F32, kind="ExternalInput")
    d_waff1T = nc.dram_tensor("waff1T", [128, 128], BF16, kind="ExternalInput")
    d_waff2T = nc.dram_tensor("waff2T", [128, 128], BF16, kind="ExternalInput")
    d_wmuT = nc.dram_tensor("wmuT", [128, 128], BF16, kind="ExternalInput")
    d_wsigT = nc.dram_tensor("wsigT", [128, 128], BF16, kind="ExternalInput")
    d_baff = nc.dram_tensor("baff", [128, 1], F32, kind="ExternalInput")
    d_bmu = nc.dram_tensor("bmu", [128, 1], F32, kind="ExternalInput")
    d_bsig = nc.dram_tensor("bsig", [128, 1], F32, kind="ExternalInput")
    d_inf_emb = nc.dram_tensor("inf_emb", [N, D], F32, kind="ExternalInput")
    d_gsig_emb = nc.dram_tensor("gsig_emb", [N, D], F32, kind="ExternalInput")
    d_wg_aug = nc.dram_tensor("wg_aug", [N, D + 1], F32, kind="ExternalInput")
    d_wg8 = nc.dram_tensor("wg8", [128, N], FP8, kind="ExternalInput")
    d_ident = nc.dram_tensor("ident", [128, 128], F32, kind="ExternalInput")
    if with_bgen:
        NCH = (N + CHUNK - 1) // CHUNK
        d_bgen = nc.dram_tensor("bgen2d", [NCH, CHUNK], BF16, kind="ExternalInput")
    d_loss = nc.dram_tensor("loss_part", [BS], F32, kind="ExternalOutput")

    with tile.TileContext(nc) as tc:
        cpool = tc.alloc_tile_pool(name="consts", bufs=1)
        wgpool = tc.alloc_tile_pool(name="wg", bufs=1)
        epool = tc.alloc_tile_pool(name="enc", bufs=1)
        ring = tc.alloc_tile_pool(name="ring", bufs=2)
        dpool = tc.alloc_tile_pool(name="dram", bufs=1, space="DRAM")
        encps = tc.alloc_tile_pool(name="encps", bufs=2, space="PSUM")
        bigps = tc.alloc_tile_pool(name="bigps", bufs=2, space="PSUM")

        # ---- index DMAs first on SP (they gate the gather chain) ----
        ctxi = cpool.tile([128, NT], I32)
        nc.sync.dma_start(out=ctxi[:], in_=d_ctx_idx[:, :])
        xi = cpool.tile([128, M], I32)
        nc.sync.dma_start(out=xi[:], in_=d_x_idx[:, :])

        # ---- W_gen fp8 streaming load on SP, group-ordered slices ----
        wg8 = wgpool.tile([128, N], FP8)
        c0 = 0
        for w in W_SLICES:
            c1 = min(c0 + w, N)
            if c1 > c0:
                nc.sync.dma_start(out=wg8[:, c0:c1], in_=d_wg8[:, c0:c1])
            c0 = c1

        # ---- small inputs issued from the ACT sequencer (idle early) ----
        identity = cpool.tile([128, 128], F32)
        nc.scalar.dma_start(out=identity[:], in_=d_ident[:, :])
        epsT = cpool.tile([128, BS], F32)
        nc.scalar.dma_start(out=epsT[:], in_=d_epsT[:, :])
        waff1T = cpool.tile([128, 128], BF16)
        nc.scalar.dma_start(out=waff1T[:], in_=d_waff1T[:, :])
        waff2T = cpool.tile([128, 128], BF16)
        nc.scalar.dma_start(out=waff2T[:], in_=d_waff2T[:, :])
        wmuT = cpool.tile([128, 128], BF16)
        nc.scalar.dma_start(out=wmuT[:], in_=d_wmuT[:, :])
        wsigT = cpool.tile([128, 128], BF16)
        nc.scalar.dma_start(out=wsigT[:], in_=d_wsigT[:, :])
        baff = cpool.tile([128, 1], F32)
        nc.scalar.dma_start(out=baff[:], in_=d_baff[:, :])
        bmu = cpool.tile([128, 1], F32)
        nc.scalar.dma_start(out=bmu[:], in_=d_bmu[:, :])
        bsig = cpool.tile([128, 1], F32)
        nc.scalar.dma_start(out=bsig[:], in_=d_bsig[:, :])
        ctxbc = cpool.tile([128, M * C], I32)
        nc.scalar.dma_start(out=ctxbc[:], in_=d_ctx_byc[:, :])

        ones_col = cpool.tile([128, 1], F32)
        nc.vector.memset(ones_col, 1.0)
        if with_bgen:
            ones_bf = cpool.tile([1, 128], BF16)
            nc.vector.memset(ones_bf, 1.0)
        if SCHRAUD_PERIOD:
            sch_c2 = cpool.tile([128, 1], F32)
            nc.vector.memset(sch_c2, SCH_C2)

        # ---- persistent tensors ----
        ctx_nat = epool.tile([128, TPM * 128 * M], F32)   # gathered ctx rows
        cg_nat = epool.tile([128, 128 * M * 2], F32)      # center | gsig rows
        ctxT = epool.tile([128, BS * C], BF16)
        centerT = epool.tile([128, BS], BF16)
        gsigT = epool.tile([128, BS], F32)
        h3 = epool.tile([128, BS * C], BF16)
        hsum_raw = epool.tile([128, BS], F32)
        hsumT = epool.tile([128, BS], BF16)
        napre = epool.tile([128, BS], F32)
        muT = epool.tile([128, BS], F32)
        infsigT = epool.tile([128, BS], F32)
        sp_tmp = epool.tile([128, 2 * BS], F32)   # [sigma-softplus | infsig-softplus] scratch
        sigmaT = epool.tile([128, BS], F32)
        lnsig = epool.tile([128, BS], F32)
        lninf = epool.tile([128, BS], F32)
        zT = epool.tile([128, BS], F32)
        z8 = epool.tile([128, BS], FP8)
        z_nat = epool.tile([128, M * (D + 1)], F32)
        wrows = epool.tile([128, M * C * (D + 1)], F32)
        tal_bc = epool.tile([128, M * C], F32)
        talsum = epool.tile([128, M], F32)
        sums = epool.tile([128, M * NG], F32)
        tot = epool.tile([128, M], F32)
        lse = epool.tile([128, M], F32)
        kli = epool.tile([128, BS], F32)
        knum = epool.tile([128, BS], F32)
        ktmp = epool.tile([128, BS], F32)
        kl_row = epool.tile([1, BS], F32)
        kl_rt = epool.tile([128, M], F32)
        loss_sb = epool.tile([128, M], F32)
        tal_scratch = epool.tile([128, D + 1], F32)

        kl_d = dpool.tile([BS], F32)

        # ================= helpers =================
        def gather(dst, src_dram, idx_col):
            nc.gpsimd.indirect_dma_start(
                out=dst, out_offset=None, in_=src_dram[:, :],
                in_offset=IndirectOffsetOnAxis(ap=idx_col, axis=0))

        def transpose_copy(nat_cols, dst_cols):
            ps = encps.tile([128, CHUNK], F32, tag="e", name="tps")
            nc.tensor.transpose(out=ps[:, :128], in_=nat_cols,
                                identity=identity[:, :])
            nc.vector.tensor_copy(out=dst_cols, in_=ps[:, :128])

        def emit_gathers(m):
            # ctx tiles then center for chunk m (order = criticality)
            for t in range(TPM):
                a0 = (m * TPM + t) * 128
                gather(ctx_nat[:, a0:a0 + 128], d_inf_emb,
                       ctxi[:, m * TPM + t:m * TPM + t + 1])
            gather(cg_nat[:, m * 128:(m + 1) * 128], d_inf_emb,
                   xi[:, m:m + 1])

        def emit_transposes(m, t):
            a0 = (m * TPM + t) * 128
            transpose_copy(ctx_nat[:, a0:a0 + 128], ctxT[:, a0:a0 + 128])

        def emit_center_transpose(m):
            transpose_copy(cg_nat[:, m * 128:(m + 1) * 128],
                           centerT[:, m * 128:(m + 1) * 128])

        def emit_encoder_a(m):
            """apre/bpre matmuls + fused relu-sum (PE + DVE)."""
            s0 = m * 128          # sample col offset
            f0 = m * TPM * 128    # flat (b,c) col offset
            apre_ps = encps.tile([128, CHUNK], F32, tag="e", name="apre")
            nc.tensor.matmul(out=apre_ps[:, :128], lhsT=waff1T[:, :],
                             rhs=centerT[:, s0:s0 + 128], start=True, stop=True)
            # napre = -(apre_psum + baff)
            nc.vector.scalar_tensor_tensor(
                out=napre[:, s0:s0 + 128], in0=apre_ps[:, :128], scalar=-1.0,
                in1=baff[:, 0:1].to_broadcast([128, 128]),
                op0=ALU.mult, op1=ALU.subtract)
            # bpre in C-aligned chunks of 510 = 51 samples
            BCH = 51 * C
            for j0 in range(0, TPM * 128, BCH):
                j1 = min(j0 + BCH, TPM * 128)
                nb = (j1 - j0) // C
                b0 = j0 // C
                bp = encps.tile([128, CHUNK], F32, tag="e", name="bp")
                nc.tensor.matmul(out=bp[:, :j1 - j0], lhsT=waff2T[:, :],
                                 rhs=ctxT[:, f0 + j0:f0 + j1],
                                 start=True, stop=True)
                # h = max(bpre, napre[b]) ; relu(a+b) = max(b,-a)+a
                nc.vector.tensor_tensor(
                    out=h3[:, f0 + j0:f0 + j1].rearrange("p (b c) -> p b c", c=C),
                    in0=bp[:, :j1 - j0].rearrange("p (b c) -> p b c", c=C),
                    in1=napre[:, s0 + b0:s0 + b0 + nb].to_broadcast([128, nb, C]),
                    op=ALU.max)
                nc.vector.reduce_sum(
                    out=hsum_raw[:, s0 + b0:s0 + b0 + nb],
                    in_=h3[:, f0 + j0:f0 + j1].rearrange("p (b c) -> p b c", c=C),
                    axis=AXX)
            # hsum = hsum_raw - C*napre  (= sum_c max + C*apre)
            nc.vector.scalar_tensor_tensor(
                out=hsumT[:, s0:s0 + 128], in0=napre[:, s0:s0 + 128],
                scalar=-float(C), in1=hsum_raw[:, s0:s0 + 128],
                op0=ALU.mult, op1=ALU.add)

        def emit_encoder_b(m):
            """mu/sig matmuls, softplus, z, z8 (PE + DVE + ACT)."""
            s0 = m * 128
            mu_ps = encps.tile([128, CHUNK], F32, tag="e", name="mups")
            nc.tensor.matmul(out=mu_ps[:, :128], lhsT=wmuT[:, :],
                             rhs=hsumT[:, s0:s0 + 128], start=True, stop=True)
            nc.vector.tensor_tensor(
                out=muT[:, s0:s0 + 128], in0=mu_ps[:, :128],
                in1=bmu[:, 0:1].to_broadcast([128, 128]), op=ALU.add)
            sig_ps = encps.tile([128, CHUNK], F32, tag="e", name="sigps")
            nc.tensor.matmul(out=sig_ps[:, :128], lhsT=wsigT[:, :],
                             rhs=hsumT[:, s0:s0 + 128], start=True, stop=True)
            # softplus = ln(1+exp(x+bsig))
            spc = sp_tmp[:, BS + s0:BS + s0 + 128]
            nc.scalar.activation(out=spc, in_=sig_ps[:, :128],
                                 func=AF.Exp, bias=bsig[:, 0:1], scale=1.0)
            nc.vector.tensor_scalar_add(out=spc, in0=spc, scalar1=1.0)
            nc.scalar.activation(out=infsigT[:, s0:s0 + 128], in_=spc, func=AF.Ln)
            # z = mu + eps * infsig
            nc.vector.tensor_tensor(out=zT[:, s0:s0 + 128],
                                    in0=epsT[:, s0:s0 + 128],
                                    in1=infsigT[:, s0:s0 + 128], op=ALU.mult)
            nc.vector.tensor_tensor(out=zT[:, s0:s0 + 128],
                                    in0=zT[:, s0:s0 + 128],
                                    in1=muT[:, s0:s0 + 128], op=ALU.add)
            nc.vector.tensor_scalar_mul(out=z8[:, s0:s0 + 128],
                                        in0=zT[:, s0:s0 + 128],
                                        scalar1=1.0 / WSCALE)

        def emit_znat(m):
            ps = encps.tile([128, CHUNK], F32, tag="e", name="zps")
            nc.tensor.transpose(out=ps[:, :128], in_=zT[:, m * 128:(m + 1) * 128],
                                identity=identity[:, :])
            a0 = m * (D + 1)
            nc.vector.tensor_copy(out=z_nat[:, a0:a0 + D], in_=ps[:, :128])
            nc.vector.tensor_copy(out=z_nat[:, a0 + D:a0 + D + 1],
                                  in_=ones_col[:, :])

        def emit_group(m, g):
            c0 = g * GROUP
            gw = min(GROUP, N - c0)
            ps = bigps.tile([128, GROUP], F32, tag="big", name="gps")
            for j0 in range(0, gw, CHUNK):
                j1 = min(j0 + CHUNK, gw)
                nc.tensor.matmul(out=ps[:, j0:j1],
                                 lhsT=z8[:, m * 128:(m + 1) * 128],
                                 rhs=wg8[:, c0 + j0:c0 + j1],
                                 start=True, stop=not with_bgen)
                if with_bgen:
                    bg = ring.tile([1, CHUNK], BF16, tag="bg", name="bg")
                    nc.sync.dma_start(
                        out=bg[:, :j1 - j0],
                        in_=d_bgen[(c0 + j0) // CHUNK, :j1 - j0]
                        .rearrange("(a b) -> a b", a=1))
                    nc.tensor.matmul(out=ps[:, j0:j1], lhsT=ones_bf[:, :],
                                     rhs=bg[:, :j1 - j0], start=False, stop=True)
            col = m * NG + g
            if SCHRAUD_PERIOD and (g % SCHRAUD_PERIOD) == (SCHRAUD_PERIOD - 1):
                yi = ring.tile([128, GROUP], I16, tag="yi", name="yi")
                nc.vector.scalar_tensor_tensor(
                    out=yi[:, :gw], in0=ps[:, :gw], scalar=SCH_C1,
                    in1=sch_c2[:, 0:1].to_broadcast([128, gw]),
                    op0=ALU.mult, op1=ALU.add)
                nc.vector.tensor_reduce(out=sums[:, col:col + 1],
                                        in_=yi[:, :gw].bitcast(BF16),
                                        axis=AXX, op=ALU.add)
            else:
                eo = ring.tile([128, GROUP], BF16, tag="eo", name="eo")
                nc.scalar.activation(out=eo[:, :gw], in_=ps[:, :gw], func=AF.Exp,
                                     accum_out=sums[:, col:col + 1])

        def emit_gsig(m):
            gather(cg_nat[:, (2 + m) * 128:(3 + m) * 128], d_gsig_emb,
                   xi[:, m:m + 1])

        def emit_gsig_transpose(m):
            transpose_copy(cg_nat[:, (2 + m) * 128:(3 + m) * 128],
                           gsigT[:, m * 128:(m + 1) * 128])

        def emit_kl_act():
            # sigma = softplus(gsig); lnsig = ln(sigma); lninf = ln(infsig)
            nc.scalar.activation(out=sp_tmp[:, :BS], in_=gsigT[:, :], func=AF.Exp)
            nc.vector.tensor_scalar_add(out=sp_tmp[:, :BS], in0=sp_tmp[:, :BS],
                                        scalar1=1.0)
            nc.scalar.activation(out=sigmaT[:, :], in_=sp_tmp[:, :BS], func=AF.Ln)
            nc.scalar.activation(out=lnsig[:, :], in_=sigmaT[:, :], func=AF.Ln)
            nc.scalar.activation(out=lninf[:, :], in_=infsigT[:, :], func=AF.Ln)

        def emit_kl_dve():
            # kli = lnsig - lninf + (infsig^2 + (mu-sigma)^2)/(2 sigma^2) - 0.5
            nc.vector.tensor_tensor(out=kli[:, :], in0=lnsig[:, :],
                                    in1=lninf[:, :], op=ALU.subtract)
            nc.vector.tensor_tensor(out=knum[:, :], in0=muT[:, :],
                                    in1=sigmaT[:, :], op=ALU.subtract)
            nc.vector.tensor_tensor(out=knum[:, :], in0=knum[:, :],
                                    in1=knum[:, :], op=ALU.mult)
            nc.vector.tensor_tensor(out=ktmp[:, :], in0=infsigT[:, :],
                                    in1=infsigT[:, :], op=ALU.mult)
            nc.vector.tensor_tensor(out=knum[:, :], in0=knum[:, :],
                                    in1=ktmp[:, :], op=ALU.add)
            nc.vector.tensor_tensor(out=ktmp[:, :], in0=sigmaT[:, :],
                                    in1=sigmaT[:, :], op=ALU.mult)
            nc.vector.tensor_scalar_mul(out=ktmp[:, :], in0=ktmp[:, :],
                                        scalar1=2.0)
            nc.vector.reciprocal(out=ktmp[:, :], in_=ktmp[:, :])
            nc.vector.tensor_tensor(out=knum[:, :], in0=knum[:, :],
                                    in1=ktmp[:, :], op=ALU.mult)
            nc.vector.tensor_tensor(out=kli[:, :], in0=kli[:, :],
                                    in1=knum[:, :], op=ALU.add)
            nc.vector.tensor_scalar_add(out=kli[:, :], in0=kli[:, :],
                                        scalar1=-0.5)

        def emit_kl_reduce():
            kl_ps = encps.tile([1, CHUNK], F32, tag="e", name="klps")
            nc.tensor.matmul(out=kl_ps[:1, :BS], lhsT=ones_col[:, :],
                             rhs=kli[:, :], start=True, stop=True)
            nc.vector.tensor_copy(out=kl_row[:, :], in_=kl_ps[:1, :BS])
            nc.sync.dma_start(out=kl_d[:].rearrange("(a b) -> a b", a=1),
                              in_=kl_row[:, :])
            nc.sync.dma_start(out=kl_rt[:, :],
                              in_=kl_d[:].rearrange("(m p) -> p m", p=128))

        def emit_wrows_gathers():
            for t in range(M * C):
                gather(wrows[:, t * (D + 1):(t + 1) * (D + 1)], d_wg_aug,
                       ctxbc[:, t:t + 1])

        def emit_tal():
            for m in range(M):
                a0 = m * (D + 1)
                for c in range(C):
                    t = m * C + c
                    w0 = t * (D + 1)
                    nc.vector.tensor_tensor_reduce(
                        out=tal_scratch[:, :], in0=wrows[:, w0:w0 + D + 1],
                        in1=z_nat[:, a0:a0 + D + 1], scale=1.0, scalar=0.0,
                        op0=ALU.mult, op1=ALU.add,
                        accum_out=tal_bc[:, t:t + 1])
            for m in range(M):
                nc.vector.reduce_sum(out=talsum[:, m:m + 1],
                                     in_=tal_bc[:, m * C:(m + 1) * C], axis=AXX)

        def emit_epilogue():
            for m in range(M):
                nc.vector.reduce_sum(out=tot[:, m:m + 1],
                                     in_=sums[:, m * NG:(m + 1) * NG], axis=AXX)
            nc.scalar.activation(out=lse[:, :], in_=tot[:, :], func=AF.Ln)
            nc.vector.tensor_tensor(out=loss_sb[:, :], in0=kl_rt[:, :],
                                    in1=talsum[:, :], op=ALU.subtract)
            nc.vector.scalar_tensor_tensor(
                out=loss_sb[:, :], in0=lse[:, :], scalar=float(C),
                in1=loss_sb[:, :], op0=ALU.mult, op1=ALU.add)
            nc.sync.dma_start(out=d_loss[:].rearrange("(m p) -> p m", p=128),
                              in_=loss_sb[:, :])

        # ================= schedule =================
        # gathers: m0 (critical) -> m1 -> gsig -> wrows, all serial on GpSimd
        emit_gathers(0)
        emit_gathers(1)
        emit_gsig(0)
        emit_gsig(1)
        emit_wrows_gathers()

        # m0 encoder (runs as gathers land)
        for t in range(TPM):
            emit_transposes(0, t)
        emit_center_transpose(0)
        emit_encoder_a(0)
        emit_encoder_b(0)
        emit_znat(0)

        # m0 streaming train, with m1 encoder interleaved by emission order
        for g in range(NG):
            emit_group(0, g)
            if g < TPM:                      # g = 0..9: m1 ctx transposes
                emit_transposes(1, g)
            elif g == TPM:                   # g = 10
                emit_center_transpose(1)
            elif g == TPM + 1:               # g = 11
                emit_encoder_a(1)
            elif g == TPM + 2:               # g = 12
                emit_encoder_b(1)
                emit_znat(1)
            elif g == TPM + 4:               # g = 14
                emit_gsig_transpose(0)
                emit_gsig_transpose(1)

        # between trains: m1 softplus already done; KL Ln's + DVE math
        emit_kl_act()
        emit_kl_dve()

        # m1 streaming train; kl partition-reduce + roundtrip injected after g2
        for g in range(NG):
            emit_group(1, g)
            if g == 2:
                emit_kl_reduce()
            elif g == 4:
                emit_tal()

        emit_epilogue()

        bigps.release()
        encps.release()
        dpool.release()
        ring.release()
        epool.release()
        wgpool.release()
        cpool.release()

    nc.compile()
    return nc


def _prep_inputs(x_batch, context_words_batch, eps, inf_emb, W_aff, b_aff,
                 W_mu, b_mu, W_sig, b_sig, gen_sigma_emb, W_gen, b_gen,
                 with_bgen):
    f32 = lambda a: np.ascontiguousarray(np.asarray(a, dtype=np.float32))
    bf16 = lambda a: np.ascontiguousarray(
        np.asarray(a, dtype=np.float32).astype(ml_dtypes.bfloat16))
    x_batch = np.asarray(x_batch, dtype=np.int32)
    ctx = np.asarray(context_words_batch, dtype=np.int32)
    eps = f32(eps)
    W_aff = np.asarray(W_aff, dtype=np.float32)
    inf_emb, gen_sigma_emb = f32(inf_emb), f32(gen_sigma_emb)
    W_gen = np.asarray(W_gen, dtype=np.float32)
    b_gen = np.asarray(b_gen, dtype=np.float32)

    shared = {
        "waff1T": bf16(W_aff[:, :D].T),
        "waff2T": bf16(W_aff[:, D:].T),
        "wmuT": bf16(np.asarray(W_mu, dtype=np.float32).T),
        "wsigT": bf16(np.asarray(W_sig, dtype=np.float32).T),
        "baff": f32(np.asarray(b_aff).reshape(D, 1)),
        "bmu": f32(np.asarray(b_mu).reshape(D, 1)),
        "bsig": f32(np.asarray(b_sig).reshape(D, 1)),
        "inf_emb": inf_emb,
        "gsig_emb": gen_sigma_emb,
        "wg_aug": np.ascontiguousarray(
            np.concatenate([W_gen, b_gen.reshape(N, 1)], axis=1)),
        "wg8": np.ascontiguousarray(
            (W_gen.T * WSCALE).astype(ml_dtypes.float8_e4m3)),
        "ident": np.eye(128, dtype=np.float32),
    }
    if with_bgen:
        NCH = (N + CHUNK - 1) // CHUNK
        bg = np.zeros((NCH * CHUNK,), dtype=ml_dtypes.bfloat16)
        bg[:N] = b_gen.astype(ml_dtypes.bfloat16)
        shared["bgen2d"] = bg.reshape(NCH, CHUNK)

    in_maps = []
    for s in range(NCORES):
        lo, hi = s * BS, (s + 1) * BS
        csh = ctx[lo:hi]                      # [BS, C]
        m = dict(shared)
        m["ctx_idx"] = np.ascontiguousarray(
            csh.reshape(BS * C).reshape(NT, 128).T)
        m["ctx_byc"] = np.ascontiguousarray(
            np.concatenate([csh[k * 128:(k + 1) * 128, :] for k in range(M)],
                           axis=1))
        m["x_idx"] = np.ascontiguousarray(x_batch[lo:hi].reshape(M, 128).T)
        m["epsT"] = np.ascontiguousarray(eps[lo:hi].T)
        in_maps.append(m)
    return in_maps


def kernel(x_batch, context_words_batch, eps, inf_emb, W_aff, b_aff,
           W_mu, b_mu, W_sig, b_sig, gen_sigma_emb, W_gen, b_gen,
           trace=False):
    with_bgen = bool(np.any(np.asarray(b_gen) != 0))
    if with_bgen not in _CACHE:
        _CACHE[with_bgen] = _build(with_bgen)
    nc = _CACHE[with_bgen]

    in_maps = _prep_inputs(x_batch, context_words_batch, eps, inf_emb, W_aff,
                           b_aff, W_mu, b_mu, W_sig, b_sig, gen_sigma_emb,
                           W_gen, b_gen, with_bgen)
    res = run_bass_kernel_spmd(nc, in_maps, core_ids=list(range(NCORES)),
                               trace=trace)
    parts = [res.results[s]["loss_part"] for s in range(NCORES)]
    loss = np.concatenate(parts).astype(np.float64).mean()
    out = np.float32(loss)
    if trace:
        kernel.last_results = res
    return out


# revision 4
# speedup vs baseline: 1.1213x; 1.1119x over previous
"""Trainium2 Bass kernel for the BayesianSkipgram loss.

Strategy (8 NeuronCores, batch-sharded, no collectives):
  - Each core computes the per-sample loss for its 256-sample batch shard.
  - The dominant cost is sum(exp(logits)) over the 50257-vocab: the ACT
    engine's Exp throughput (0.833ns/elem) is the roofline. Everything else
    is scheduled to hide under the two 33-group EXP trains (one per
    128-sample chunk m).
  - Pipelined start: chunk m0's gathers + encoder run first so the m0 EXP
    train starts at ~24us; m1's gathers/encoder/KL/take-along all overlap
    the m0 train via careful per-engine emission interleaving.
  - W_gen^T is pre-scaled x8 and cast to fp8e4 on host (6.4MB), streamed in
    28 column-slices so group 0 lands before the first matmul. z is scaled
    1/8 to fp8 for the logits matmul (PSUM accumulates f32); logit rms
    error ~4% of logit sigma, irrelevant for logsumexp at 2e-2 tolerance.
  - PSUM: 6 banks = 2 x [128,1536] streaming tiles, 2 banks = encoder ring
    (transposes + small matmuls), so encoder m1 overlaps the m0 train.
  - take_along(logp) is exact: fp32 indirect-DMA row gathers of
    [W_gen | b_gen] dotted against fp32 z (tensor_tensor_reduce).
  - Optionally (SCHRAUD_PERIOD) a fraction of vocab groups compute
    sum-exp on the DVE via a Schraudolph bit-trick exp (int16 + bitcast
    bf16), offloading the ACT engine.
  - Host combines the 8x[256] per-sample losses with a mean.
"""

import numpy as np
import ml_dtypes

import concourse.bass as bass
import concourse.mybir as mybir
import concourse.tile as tile
from concourse import bacc
from concourse.bass import IndirectOffsetOnAxis
from concourse.bass_utils import run_bass_kernel_spmd

F32 = mybir.dt.float32
BF16 = mybir.dt.bfloat16
FP8 = mybir.dt.float8e4
I16 = mybir.dt.int16
I32 = mybir.dt.int32
AF = mybir.ActivationFunctionType
ALU = mybir.AluOpType
AXX = mybir.AxisListType.X

N = 50257      # vocab
D = 128        # embedding dim
B = 2048       # total batch
C = 10         # context size
NCORES = 8
BS = B // NCORES          # 256 samples per core
M = BS // 128             # 2 partition chunks of samples
NT = BS * C // 128        # 20 ctx gather tiles (10 per m)
TPM = NT // M             # 10
GROUP = 1536              # streaming group (3 psum banks)
NG = (N + GROUP - 1) // GROUP        # 33 groups
CHUNK = 512               # matmul free-dim
WSCALE = 8.0              # W_gen pre-scale (z scaled 1/WSCALE)

# Schraudolph DVE-exp offload: every SCHRAUD_PERIOD-th group (within an m)
# is computed on the DVE instead of ACT. 0 = disabled (all exact ACT exp).
SCHRAUD_PERIOD = 0
SCH_C1 = 184.6650390625   # 2^7 * log2(e)
SCH_C2 = 16250.25         # calibrated: 127*2^7 minus mean-centering tweak

# W load slicing: first fine slices for fast group-0 arrival, then coarse.
W_SLICES = [1024] * 6 + [2048] * 22   # 6*1024 + 22*2048 = 51200 >= N

_CACHE = {}


def _patch_act_tables():
    """Keep Exp/Ln/Identity/Copy only in natural_log_exp_and_others so the
    table-load inserter uses one set for the whole kernel."""
    import concourse.bacc as _bacc_mod
    import concourse.hw_specs as _hws
    if getattr(_bacc_mod, "_ant_act_tables_patched", False):
        return
    _orig = _hws.get_activation_tables
    _ours = {AF.Exp, AF.Ln, AF.Identity, AF.Copy}

    def _filtered(arch):
        tabs = _orig(arch)
        out = {}
        for name, funcs in tabs.items():
            if name == "natural_log_exp_and_others" or not (_ours & funcs):
                out[name] = funcs
            else:
                out[name] = funcs - _ours
        return out

    _bacc_mod.get_activation_tables = _filtered
    _bacc_mod._ant_act_tables_patched = True


def _build(with_bgen: bool):
    _patch_act_tables()
    nc = bacc.Bacc("TRN2", target_bir_lowering=False, debug=False)

    # ---------------- DRAM I/O ----------------
    d_ctx_idx = nc.dram_tensor("ctx_idx", [128, NT], I32, kind="ExternalInput")
    d_ctx_byc = nc.dram_tensor("ctx_byc", [128, M * C], I32, kind="ExternalInput")
    d_x_idx = nc.dram_tensor("x_idx", [128, M], I32, kind="ExternalInput")
    d_epsT = nc.dram_tensor("epsT", [128, BS], F32, kind="ExternalInput")
    d_waff1T = nc.dram_tensor("waff1T", [128, 128], BF16, kind="ExternalInput")
    d_waff2T = nc.dram_tensor("waff2T", [128, 128], BF16, kind="ExternalInput")
    d_wmuT = nc.dram_tensor("wmuT", [128, 128], BF16, kind="ExternalInput")
    d_wsigT = nc.dram_tensor("wsigT", [128, 128], BF16, kind="ExternalInput")
    d_baff = nc.dram_tensor("baff", [128, 1], F32, kind="ExternalInput")
    d_bmu = nc.dram_tensor("bmu", [128, 1], F32, kind="ExternalInput")
    d_bsig = nc.dram_tensor("bsig", [128, 1], F32, kind="ExternalInput")
    d_inf_emb = nc.dram_tensor("inf_emb", [N, D], F32, kind="ExternalInput")
    d_gsig_emb = nc.dram_tensor("gsig_emb", [N, D], F32, kind="ExternalInput")
    d_wg_aug = nc.dram_tensor("wg_aug", [N, D + 1], F32, kind="ExternalInput")
    d_wg8 = nc.dram_tensor("wg8", [128, N], FP8, kind="ExternalInput")
    d_ident = nc.dram_tensor("ident", [128, 128], F32, kind="ExternalInput")
    if with_bgen:
        NCH = (N + CHUNK - 1) // CHUNK
        d_bgen = nc.dram_tensor("bgen2d", [NCH, CHUNK], BF16, kind="ExternalInput")
    d_loss = nc.dram_tensor("loss_part", [BS], F32, kind="ExternalOutput")

    with tile.TileContext(nc) as tc:
        cpool = tc.alloc_tile_pool(name="consts", bufs=1)
        wgpool = tc.alloc_tile_pool(name="wg", bufs=1)
        epool = tc.alloc_tile_pool(name="enc", bufs=1)
        ring = tc.alloc_tile_pool(name="ring", bufs=2)
        dpool = tc.alloc_tile_pool(name="dram", bufs=1, space="DRAM")
        encps = tc.alloc_tile_pool(name="encps", bufs=2, space="PSUM")
        bigps = tc.alloc_tile_pool(name="bigps", bufs=2, space="PSUM")

        # ---- index DMAs first on SP (they gate the gather chain) ----
        ctxi = cpool.tile([128, NT], I32)
        nc.sync.dma_start(out=ctxi[:], in_=d_ctx_idx[:, :])
        xi = cpool.tile([128, M], I32)
        nc.sync.dma_start(out=xi[:], in_=d_x_idx[:, :])

        # ---- W_gen fp8 streaming load on SP, group-ordered slices ----
        wg8 = wgpool.tile([128, N], FP8)
        c0 = 0
        for w in W_SLICES:
            c1 = min(c0 + w, N)
            if c1 > c0:
                nc.sync.dma_start(out=wg8[:, c0:c1], in_=d_wg8[:, c0:c1])
            c0 = c1

        # ---- small inputs issued from the ACT sequencer (idle early) ----
        identity = cpool.tile([128, 128], F32)
        nc.scalar.dma_start(out=identity[:], in_=d_ident[:, :])
        epsT = cpool.tile([128, BS], F32)
        nc.scalar.dma_start(out=epsT[:], in_=d_epsT[:, :])
        waff1T = cpool.tile([128, 128], BF16)
        nc.scalar.dma_start(out=waff1T[:], in_=d_waff1T[:, :])
        waff2T = cpool.tile([128, 128], BF16)
        nc.scalar.dma_start(out=waff2T[:], in_=d_waff2T[:, :])
        wmuT = cpool.tile([128, 128], BF16)
        nc.scalar.dma_start(out=wmuT[:], in_=d_wmuT[:, :])
        wsigT = cpool.tile([128, 128], BF16)
        nc.scalar.dma_start(out=wsigT[:], in_=d_wsigT[:, :])
        baff = cpool.tile([128, 1], F32)
        nc.scalar.dma_start(out=baff[:], in_=d_baff[:, :])
        bmu = cpool.tile([128, 1], F32)
        nc.scalar.dma_start(out=bmu[:], in_=d_bmu[:, :])
        bsig = cpool.tile([128, 1], F32)
        nc.scalar.dma_start(out=bsig[:], in_=d_bsig[:, :])
        ctxbc = cpool.tile([128, M * C], I32)
        nc.scalar.dma_start(out=ctxbc[:], in_=d_ctx_byc[:, :])

        ones_col = cpool.tile([128, 1], F32)
        nc.vector.memset(ones_col, 1.0)
        if with_bgen:
            ones_bf = cpool.tile([1, 128], BF16)
            nc.vector.memset(ones_bf, 1.0)
        if SCHRAUD_PERIOD:
            sch_c2 = cpool.tile([128, 1], F32)
            nc.vector.memset(sch_c2, SCH_C2)

        # ---- persistent tensors ----
        ctx_nat = epool.tile([128, TPM * 128 * M], F32)   # gathered ctx rows
        cg_nat = epool.tile([128, 128 * M * 2], F32)      # center | gsig rows
        ctxT = epool.tile([128, BS * C], BF16)
        centerT = epool.tile([128, BS], BF16)
        gsigT = epool.tile([128, BS], F32)
        h3 = epool.tile([128, BS * C], BF16)
        hsum_raw = epool.tile([128, BS], F32)
        hsumT = epool.tile([128, BS], BF16)
        napre = epool.tile([128, BS], F32)
        muT = epool.tile([128, BS], F32)
        infsigT = epool.tile([128, BS], F32)
        sp_tmp = epool.tile([128, 2 * BS], F32)   # [sigma-softplus | infsig-softplus] scratch
        sigmaT = epool.tile([128, BS], F32)
        lnsig = epool.tile([128, BS], F32)
        lninf = epool.tile([128, BS], F32)
        zT = epool.tile([128, BS], F32)
        z8 = epool.tile([128, BS], FP8)
        z_nat = epool.tile([128, M * (D + 1)], F32)
        wrows = epool.tile([128, M * C * (D + 1)], F32)
        tal_bc = epool.tile([128, M * C], F32)
        talsum = epool.tile([128, M], F32)
        sums = epool.tile([128, M * NG], F32)
        tot = epool.tile([128, M], F32)
        lse = epool.tile([128, M], F32)
        kli = epool.tile([128, BS], F32)
        knum = epool.tile([128, BS], F32)
        ktmp = epool.tile([128, BS], F32)
        kl_row = epool.tile([1, BS], F32)
        kl_rt = epool.tile([128, M], F32)
        loss_sb = epool.tile([128, M], F32)
        tal_scratch = epool.tile([128, D + 1], F32)

        kl_d = dpool.tile([BS], F32)

        # ================= helpers =================
        def gather(dst, src_dram, idx_col):
            nc.gpsimd.indirect_dma_start(
                out=dst, out_offset=None, in_=src_dram[:, :],
                in_offset=IndirectOffsetOnAxis(ap=idx_col, axis=0))

        def transpose_copy(nat_cols, dst_cols):
            ps = encps.tile([128, CHUNK], F32, tag="e", name="tps")
            nc.tensor.transpose(out=ps[:, :128], in_=nat_cols,
                                identity=identity[:, :])
            nc.vector.tensor_copy(out=dst_cols, in_=ps[:, :128])

        def emit_gathers(m):
            # ctx tiles then center for chunk m (order = criticality)
            for t in range(TPM):
                a0 = (m * TPM + t) * 128
                gather(ctx_nat[:, a0:a0 + 128], d_inf_emb,
                       ctxi[:, m * TPM + t:m * TPM + t + 1])
            gather(cg_nat[:, m * 128:(m + 1) * 128], d_inf_emb,
                   xi[:, m:m + 1])

        def emit_transposes(m, t):
            a0 = (m * TPM + t) * 128
            transpose_copy(ctx_nat[:, a0:a0 + 128], ctxT[:, a0:a0 + 128])

        def emit_center_transpose(m):
            transpose_copy(cg_nat[:, m * 128:(m + 1) * 128],
                           centerT[:, m * 128:(m + 1) * 128])

        def emit_encoder_a(m):
            """apre/bpre matmuls + fused relu-sum (PE + DVE)."""
            s0 = m * 128          # sample col offset
            f0 = m * TPM * 128    # flat (b,c) col offset
            apre_ps = encps.tile([128, CHUNK], F32, tag="e", name="apre")
            nc.tensor.matmul(out=apre_ps[:, :128], lhsT=waff1T[:, :],
                             rhs=centerT[:, s0:s0 + 128], start=True, stop=True)
            # napre = -(apre_psum + baff)
            nc.vector.scalar_tensor_tensor(
                out=napre[:, s0:s0 + 128], in0=apre_ps[:, :128], scalar=-1.0,
                in1=baff[:, 0:1].to_broadcast([128, 128]),
                op0=ALU.mult, op1=ALU.subtract)
            # bpre in C-aligned chunks of 510 = 51 samples
            BCH = 51 * C
            for j0 in range(0, TPM * 128, BCH):
                j1 = min(j0 + BCH, TPM * 128)
                nb = (j1 - j0) // C
                b0 = j0 // C
                bp = encps.tile([128, CHUNK], F32, tag="e", name="bp")
                nc.tensor.matmul(out=bp[:, :j1 - j0], lhsT=waff2T[:, :],
                                 rhs=ctxT[:, f0 + j0:f0 + j1],
                                 start=True, stop=True)
                # h = max(bpre, napre[b]) ; relu(a+b) = max(b,-a)+a
                nc.vector.tensor_tensor(
                    out=h3[:, f0 + j0:f0 + j1].rearrange("p (b c) -> p b c", c=C),
                    in0=bp[:, :j1 - j0].rearrange("p (b c) -> p b c", c=C),
                    in1=napre[:, s0 + b0:s0 + b0 + nb].to_broadcast([128, nb, C]),
                    op=ALU.max)
                nc.vector.reduce_sum(
                    out=hsum_raw[:, s0 + b0:s0 + b0 + nb],
                    in_=h3[:, f0 + j0:f0 + j1].rearrange("p (b c) -> p b c", c=C),
                    axis=AXX)
            # hsum = hsum_raw - C*napre  (= sum_c max + C*apre)
            nc.vector.scalar_tensor_tensor(
                out=hsumT[:, s0:s0 + 128], in0=napre[:, s0:s0 + 128],
                scalar=-float(C), in1=hsum_raw[:, s0:s0 + 128],
                op0=ALU.mult, op1=ALU.add)

        def emit_encoder_b(m):
            """mu/sig matmuls, softplus, z, z8 (PE + DVE + ACT)."""
            s0 = m * 128
            mu_ps = encps.tile([128, CHUNK], F32, tag="e", name="mups")
            nc.tensor.matmul(out=mu_ps[:, :128], lhsT=wmuT[:, :],
                             rhs=hsumT[:, s0:s0 + 128], start=True, stop=True)
            nc.vector.tensor_tensor(
                out=muT[:, s0:s0 + 128], in0=mu_ps[:, :128],
                in1=bmu[:, 0:1].to_broadcast([128, 128]), op=ALU.add)
            sig_ps = encps.tile([128, CHUNK], F32, tag="e", name="sigps")
            nc.tensor.matmul(out=sig_ps[:, :128], lhsT=wsigT[:, :],
                             rhs=hsumT[:, s0:s0 + 128], start=True, stop=True)
            # softplus = ln(1+exp(x+bsig))
            spc = sp_tmp[:, BS + s0:BS + s0 + 128]
            nc.scalar.activation(out=spc, in_=sig_ps[:, :128],
                                 func=AF.Exp, bias=bsig[:, 0:1], scale=1.0)
            nc.vector.tensor_scalar_add(out=spc, in0=spc, scalar1=1.0)
            nc.scalar.activation(out=infsigT[:, s0:s0 + 128], in_=spc, func=AF.Ln)
            # z = mu + eps * infsig
            nc.vector.tensor_tensor(out=zT[:, s0:s0 + 128],
                                    in0=epsT[:, s0:s0 + 128],
                                    in1=infsigT[:, s0:s0 + 128], op=ALU.mult)
            nc.vector.tensor_tensor(out=zT[:, s0:s0 + 128],
                                    in0=zT[:, s0:s0 + 128],
                                    in1=muT[:, s0:s0 + 128], op=ALU.add)
            nc.vector.tensor_scalar_mul(out=z8[:, s0:s0 + 128],
                                        in0=zT[:, s0:s0 + 128],
                                        scalar1=1.0 / WSCALE)

        def emit_znat(m):
            ps = encps.tile([128, CHUNK], F32, tag="e", name="zps")
            nc.tensor.transpose(out=ps[:, :128], in_=zT[:, m * 128:(m + 1) * 128],
                                identity=identity[:, :])
            a0 = m * (D + 1)
            nc.vector.tensor_copy(out=z_nat[:, a0:a0 + D], in_=ps[:, :128])
            nc.vector.tensor_copy(out=z_nat[:, a0 + D:a0 + D + 1],
                                  in_=ones_col[:, :])

        def emit_group(m, g):
            c0 = g * GROUP
            gw = min(GROUP, N - c0)
            ps = bigps.tile([128, GROUP], F32, tag="big", name="gps")
            for j0 in range(0, gw, CHUNK):
                j1 = min(j0 + CHUNK, gw)
                nc.tensor.matmul(out=ps[:, j0:j1],
                                 lhsT=z8[:, m * 128:(m + 1) * 128],
                                 rhs=wg8[:, c0 + j0:c0 + j1],
                                 start=True, stop=not with_bgen)
                if with_bgen:
                    bg = ring.tile([1, CHUNK], BF16, tag="bg", name="bg")
                    nc.sync.dma_start(
                        out=bg[:, :j1 - j0],
                        in_=d_bgen[(c0 + j0) // CHUNK, :j1 - j0]
                        .rearrange("(a b) -> a b", a=1))
                    nc.tensor.matmul(out=ps[:, j0:j1], lhsT=ones_bf[:, :],
                                     rhs=bg[:, :j1 - j0], start=False, stop=True)
            col = m * NG + g
            if SCHRAUD_PERIOD and (g % SCHRAUD_PERIOD) == (SCHRAUD_PERIOD - 1):
                yi = ring.tile([128, GROUP], I16, tag="yi", name="yi")
                nc.vector.scalar_tensor_tensor(
                    out=yi[:, :gw], in0=ps[:, :gw], scalar=SCH_C1,
                    in1=sch_c2[:, 0:1].to_broadcast([128, gw]),
                    op0=ALU.mult, op1=ALU.add)
                nc.vector.tensor_reduce(out=sums[:, col:col + 1],
                                        in_=yi[:, :gw].bitcast(BF16),
                                        axis=AXX, op=ALU.add)
            else:
                eo = ring.tile([128, GROUP], BF16, tag="eo", name="eo")
                nc.scalar.activation(out=eo[:, :gw], in_=ps[:, :gw], func=AF.Exp,
                                     accum_out=sums[:, col:col + 1])

        def emit_gsig(m):
            gather(cg_nat[:, (2 + m) * 128:(3 + m) * 128], d_gsig_emb,
                   xi[:, m:m + 1])

        def emit_gsig_transpose(m):
            transpose_copy(cg_nat[:, (2 + m) * 128:(3 + m) * 128],
                           gsigT[:, m * 128:(m + 1) * 128])

        def emit_kl_act():
            # sigma = softplus(gsig); lnsig = ln(sigma); lninf = ln(infsig)
            nc.scalar.activation(out=sp_tmp[:, :BS], in_=gsigT[:, :], func=AF.Exp)
            nc.vector.tensor_scalar_add(out=sp_tmp[:, :BS], in0=sp_tmp[:, :BS],
                                        scalar1=1.0)
            nc.scalar.activation(out=sigmaT[:, :], in_=sp_tmp[:, :BS], func=AF.Ln)
            nc.scalar.activation(out=lnsig[:, :], in_=sigmaT[:, :], func=AF.Ln)
            nc.scalar.activation(out=lninf[:, :], in_=infsigT[:, :], func=AF.Ln)

        def emit_kl_dve():
            # kli = lnsig - lninf + (infsig^2 + (mu-sigma)^2)/(2 sigma^2) - 0.5
            nc.vector.tensor_tensor(out=kli[:, :], in0=lnsig[:, :],
                                    in1=lninf[:, :], op=ALU.subtract)
            nc.vector.tensor_tensor(out=knum[:, :], in0=muT[:, :],
                                    in1=sigmaT[:, :], op=ALU.subtract)
            nc.vector.tensor_tensor(out=knum[:, :], in0=knum[:, :],
                                    in1=knum[:, :], op=ALU.mult)
            nc.vector.tensor_tensor(out=ktmp[:, :], in0=infsigT[:, :],
                                    in1=infsigT[:, :], op=ALU.mult)
            nc.vector.tensor_tensor(out=knum[:, :], in0=knum[:, :],
                                    in1=ktmp[:, :], op=ALU.add)
            nc.vector.tensor_tensor(out=ktmp[:, :], in0=sigmaT[:, :],
                                    in1=sigmaT[:, :], op=ALU.mult)
            nc.vector.tensor_scalar_mul(out=ktmp[:, :], in0=ktmp[:, :],
                                        scalar1=2.0)
            nc.vector.reciprocal(out=ktmp[:, :], in_=ktmp[:, :])
            nc.vector.tensor_tensor(out=knum[:, :], in0=knum[:, :],
                                    in1=ktmp[:, :], op=ALU.mult)
            nc.vector.tensor_tensor(out=kli[:, :], in0=kli[:, :],
                                    in1=knum[:, :], op=ALU.add)
            nc.vector.tensor_scalar_add(out=kli[:, :], in0=kli[:, :],
                                        scalar1=-0.5)

        def emit_kl_reduce():
            kl_ps = encps.tile([1, CHUNK], F32, tag="e", name="klps")
            nc.tensor.matmul(out=kl_ps[:1, :BS], lhsT=ones_col[:, :],
                             rhs=kli[:, :], start=True, stop=True)
            nc.vector.tensor_copy(out=kl_row[:, :], in_=kl_ps[:1, :BS])
            nc.sync.dma_start(out=kl_d[:].rearrange("(a b) -> a b", a=1),
                              in_=kl_row[:, :])
            nc.sync.dma_start(out=kl_rt[:, :],
                              in_=kl_d[:].rearrange("(m p) -> p m", p=128))

        def emit_wrows_gathers():
            for t in range(M * C):
                gather(wrows[:, t * (D + 1):(t + 1) * (D + 1)], d_wg_aug,
                       ctxbc[:, t:t + 1])

        def emit_tal():
            # (tensor_tensor_reduce would fuse this but crashes real HW)
            for m in range(M):
                a0 = m * (D + 1)
                for c in range(C):
                    t = m * C + c
                    w0 = t * (D + 1)
                    nc.vector.tensor_tensor(
                        out=tal_scratch[:, :], in0=wrows[:, w0:w0 + D + 1],
                        in1=z_nat[:, a0:a0 + D + 1], op=ALU.mult)
                    nc.vector.reduce_sum(out=tal_bc[:, t:t + 1],
                                         in_=tal_scratch[:, :], axis=AXX)
            for m in range(M):
                nc.vector.reduce_sum(out=talsum[:, m:m + 1],
                                     in_=tal_bc[:, m * C:(m + 1) * C], axis=AXX)

        def emit_epilogue():
            for m in range(M):
                nc.vector.reduce_sum(out=tot[:, m:m + 1],
                                     in_=sums[:, m * NG:(m + 1) * NG], axis=AXX)
            nc.scalar.activation(out=lse[:, :], in_=tot[:, :], func=AF.Ln)
            nc.vector.tensor_tensor(out=loss_sb[:, :], in0=kl_rt[:, :],
                                    in1=talsum[:, :], op=ALU.subtract)
            nc.vector.scalar_tensor_tensor(
                out=loss_sb[:, :], in0=lse[:, :], scalar=float(C),
                in1=loss_sb[:, :], op0=ALU.mult, op1=ALU.add)
            nc.sync.dma_start(out=d_loss[:].rearrange("(m p) -> p m", p=128),
                              in_=loss_sb[:, :])

        # ================= schedule =================
        # gathers: m0 (critical) -> m1 -> gsig -> wrows, all serial on GpSimd
        emit_gathers(0)
        emit_gathers(1)
        emit_gsig(0)
        emit_gsig(1)
        emit_wrows_gathers()

        # m0 encoder (runs as gathers land)
        for t in range(TPM):
            emit_transposes(0, t)
        emit_center_transpose(0)
        emit_encoder_a(0)
        emit_encoder_b(0)
        emit_znat(0)

        # m0 streaming train, with m1 encoder interleaved by emission order
        for g in range(NG):
            emit_group(0, g)
            if g < TPM:                      # g = 0..9: m1 ctx transposes
                emit_transposes(1, g)
            elif g == TPM:                   # g = 10
                emit_center_transpose(1)
            elif g == TPM + 1:               # g = 11
                emit_encoder_a(1)
            elif g == TPM + 2:               # g = 12
                emit_encoder_b(1)
                emit_znat(1)
            elif g == TPM + 4:               # g = 14
                emit_gsig_transpose(0)
                emit_gsig_transpose(1)

        # between trains: m1 softplus already done; KL Ln's + DVE math
        emit_kl_act()
        emit_kl_dve()

        # m1 streaming train; kl partition-reduce + roundtrip injected after g2
        for g in range(NG):
            emit_group(1, g)
            if g == 2:
                emit_kl_reduce()
            elif g == 4:
                emit_tal()

        emit_epilogue()

        bigps.release()
        encps.release()
        dpool.release()
        ring.release()
        epool.release()
        wgpool.release()
        cpool.release()

    nc.compile()
    return nc


def _prep_inputs(x_batch, context_words_batch, eps, inf_emb, W_aff, b_aff,
                 W_mu, b_mu, W_sig, b_sig, gen_sigma_emb, W_gen, b_gen,
                 with_bgen):
    f32 = lambda a: np.ascontiguousarray(np.asarray(a, dtype=np.float32))
    bf16 = lambda a: np.ascontiguousarray(
        np.asarray(a, dtype=np.float32).astype(ml_dtypes.bfloat16))
    x_batch = np.asarray(x_batch, dtype=np.int32)
    ctx = np.asarray(context_words_batch, dtype=np.int32)
    eps = f32(eps)
    W_aff = np.asarray(W_aff, dtype=np.float32)
    inf_emb, gen_sigma_emb = f32(inf_emb), f32(gen_sigma_emb)
    W_gen = np.asarray(W_gen, dtype=np.float32)
    b_gen = np.asarray(b_gen, dtype=np.float32)

    shared = {
        "waff1T": bf16(W_aff[:, :D].T),
        "waff2T": bf16(W_aff[:, D:].T),
        "wmuT": bf16(np.asarray(W_mu, dtype=np.float32).T),
        "wsigT": bf16(np.asarray(W_sig, dtype=np.float32).T),
        "baff": f32(np.asarray(b_aff).reshape(D, 1)),
        "bmu": f32(np.asarray(b_mu).reshape(D, 1)),
        "bsig": f32(np.asarray(b_sig).reshape(D, 1)),
        "inf_emb": inf_emb,
        "gsig_emb": gen_sigma_emb,
        "wg_aug": np.ascontiguousarray(
            np.concatenate([W_gen, b_gen.reshape(N, 1)], axis=1)),
        "wg8": np.ascontiguousarray(
            (W_gen.T * WSCALE).astype(ml_dtypes.float8_e4m3)),
        "ident": np.eye(128, dtype=np.float32),
    }
    if with_bgen:
        NCH = (N + CHUNK - 1) // CHUNK
        bg = np.zeros((NCH * CHUNK,), dtype=ml_dtypes.bfloat16)
        bg[:N] = b_gen.astype(ml_dtypes.bfloat16)
        shared["bgen2d"] = bg.reshape(NCH, CHUNK)

    in_maps = []
    for s in range(NCORES):
        lo, hi = s * BS, (s + 1) * BS
        csh = ctx[lo:hi]                      # [BS, C]
        m = dict(shared)
        m["ctx_idx"] = np.ascontiguousarray(
            csh.reshape(BS * C).reshape(NT, 128).T)
        m["ctx_byc"] = np.ascontiguousarray(
            np.concatenate([csh[k * 128:(k + 1) * 128, :] for k in range(M)],
                           axis=1))
        m["x_idx"] = np.ascontiguousarray(x_batch[lo:hi].reshape(M, 128).T)
        m["epsT"] = np.ascontiguousarray(eps[lo:hi].T)
        in_maps.append(m)
    return in_maps


def kernel(x_batch, context_words_batch, eps, inf_emb, W_aff, b_aff,
           W_mu, b_mu, W_sig, b_sig, gen_sigma_emb, W_gen, b_gen,
           trace=False):
    with_bgen = bool(np.any(np.asarray(b_gen) != 0))
    if with_bgen not in _CACHE:
        _CACHE[with_bgen] = _build(with_bgen)
    nc = _CACHE[with_bgen]

    in_maps = _prep_inputs(x_batch, context_words_batch, eps, inf_emb, W_aff,
                           b_aff, W_mu, b_mu, W_sig, b_sig, gen_sigma_emb,
                           W_gen, b_gen, with_bgen)
    res = run_bass_kernel_spmd(nc, in_maps, core_ids=list(range(NCORES)),
                               trace=trace)
    parts = [res.results[s]["loss_part"] for s in range(NCORES)]
    loss = np.concatenate(parts).astype(np.float64).mean()
    out = np.float32(loss)
    if trace:
        kernel.last_results = res
    return out


# revision 13
# speedup vs baseline: 1.1973x; 1.0677x over previous
"""Trainium2 Bass kernel for the BayesianSkipgram loss.

Strategy (8 NeuronCores, batch-sharded, no collectives):
  - Each core computes the per-sample loss for its 256-sample batch shard.
  - The dominant cost is sum(exp(logits)) over the 50257-vocab. The ACT
    engine's Exp (0.833ns/elem, with free per-group accumulation) and a
    DVE Schraudolph bit-trick exp (scalar_tensor_tensor -> int16, bitcast
    bf16, reduce) split the vocab groups ~73/27 so both engines stream
    concurrently. Everything else hides under the two 33-group trains.
  - Pipelined start: chunk m0's center gather lands first, the encoder
    consumes ctx tiles per bpre-chunk as gathers land, so the m0 train
    starts as early as possible. m1's encoder runs as one block in the
    middle of the m0 train; KL and take-along overlap the trains.
  - W_gen^T is pre-scaled x8, cast to fp8e4 on host (6.4MB), streamed in
    28 column-slices so group 0 lands before the first matmul. z is scaled
    1/8 to fp8 for the logits matmul (PSUM accumulates f32); logit rms
    error ~4% of logit sigma, negligible for logsumexp at 2e-2 tolerance.
  - PSUM: 6 banks = 2 x [128,1536] streaming tiles, 2 banks = encoder ring.
  - take_along(logp) is exact: fp32 indirect-DMA row gathers of
    [W_gen | b_gen] dotted against fp32 z on the (otherwise idle) GpSimd.
  - Host combines the 8x[256] per-sample losses with a mean.
"""

import numpy as np
import ml_dtypes

import concourse.bass as bass
import concourse.mybir as mybir
import concourse.tile as tile
from concourse import bacc
from concourse.bass import IndirectOffsetOnAxis
from concourse.bass_utils import run_bass_kernel_spmd

F32 = mybir.dt.float32
BF16 = mybir.dt.bfloat16
FP8 = mybir.dt.float8e4
I16 = mybir.dt.int16
I32 = mybir.dt.int32
AF = mybir.ActivationFunctionType
ALU = mybir.AluOpType
AXX = mybir.AxisListType.X

N = 50257      # vocab
D = 128        # embedding dim
B = 2048       # total batch
C = 10         # context size
NCORES = 8
BS = B // NCORES          # 256 samples per core
M = BS // 128             # 2 partition chunks of samples
NT = BS * C // 128        # 20 ctx gather tiles (10 per m)
TPM = NT // M             # 10
GROUP = 1536              # streaming group (3 psum banks)
NG = (N + GROUP - 1) // GROUP        # 33 groups
CHUNK = 512               # matmul free-dim
WSCALE = 8.0              # W_gen pre-scale (z scaled 1/WSCALE)

# Schraudolph DVE-exp offload: groups with g % SCH_MOD in SCH_SET go to the
# DVE. Empty set = all exact ACT exp.
SCH_MOD = 7
SCH_SET = frozenset({2, 5})
SCH_C1 = 184.6650390625   # 2^7 * log2(e)
SCH_C2 = 16250.25         # calibrated: 127*2^7 minus mean-centering tweak

# bpre is computed in C-aligned chunks; chunk k consumes ctx tiles [lo, hi)
BCH = 51 * C              # 510 cols = 51 samples
BP_CHUNKS = [(0, 510, 0, 4), (510, 1020, 3, 8), (1020, 1280, 7, 10)]

# W load slicing: first fine slices for fast group-0 arrival, then coarse.
W_SLICES = [1024] * 6 + [2048] * 22   # 6*1024 + 22*2048 = 51200 >= N

_CACHE = {}


def _patch_act_tables():
    """Keep Exp/Ln/Identity/Copy only in natural_log_exp_and_others so the
    table-load inserter uses one set for the whole kernel."""
    import concourse.bacc as _bacc_mod
    import concourse.hw_specs as _hws
    if getattr(_bacc_mod, "_ant_act_tables_patched", False):
        return
    _orig = _hws.get_activation_tables
    _ours = {AF.Exp, AF.Ln, AF.Identity, AF.Copy}

    def _filtered(arch):
        tabs = _orig(arch)
        out = {}
        for name, funcs in tabs.items():
            if name == "natural_log_exp_and_others" or not (_ours & funcs):
                out[name] = funcs
            else:
                out[name] = funcs - _ours
        return out

    _bacc_mod.get_activation_tables = _filtered
    _bacc_mod._ant_act_tables_patched = True


def _build(with_bgen: bool):
    _patch_act_tables()
    nc = bacc.Bacc("TRN2", target_bir_lowering=False, debug=False)

    # ---------------- DRAM I/O ----------------
    d_ctx_idx = nc.dram_tensor("ctx_idx", [128, NT], I32, kind="ExternalInput")
    d_ctx_byc = nc.dram_tensor("ctx_byc", [128, M * C], I32, kind="ExternalInput")
    d_x_idx = nc.dram_tensor("x_idx", [128, M], I32, kind="ExternalInput")
    d_epsT = nc.dram_tensor("epsT", [128, BS], F32, kind="ExternalInput")
    d_waff1T = nc.dram_tensor("waff1T", [128, 128], BF16, kind="ExternalInput")
    d_waff2T = nc.dram_tensor("waff2T", [128, 128], BF16, kind="ExternalInput")
    d_wmuT = nc.dram_tensor("wmuT", [128, 128], BF16, kind="ExternalInput")
    d_wsigT = nc.dram_tensor("wsigT", [128, 128], BF16, kind="ExternalInput")
    d_baff = nc.dram_tensor("baff", [128, 1], F32, kind="ExternalInput")
    d_bmu = nc.dram_tensor("bmu", [128, 1], F32, kind="ExternalInput")
    d_bsig = nc.dram_tensor("bsig", [128, 1], F32, kind="ExternalInput")
    d_inf_bf = nc.dram_tensor("inf_bf", [N, D], BF16, kind="ExternalInput")
    d_gsig_emb = nc.dram_tensor("gsig_emb", [N, D], F32, kind="ExternalInput")
    d_wg_aug = nc.dram_tensor("wg_aug", [N, D + 1], F32, kind="ExternalInput")
    d_wg8 = nc.dram_tensor("wg8", [128, N], FP8, kind="ExternalInput")
    d_ident = nc.dram_tensor("ident", [128, 128], F32, kind="ExternalInput")
    d_ident_bf = nc.dram_tensor("ident_bf", [128, 128], BF16, kind="ExternalInput")
    if with_bgen:
        NCH = (N + CHUNK - 1) // CHUNK
        d_bgen = nc.dram_tensor("bgen2d", [NCH, CHUNK], BF16, kind="ExternalInput")
    d_loss = nc.dram_tensor("loss_part", [BS], F32, kind="ExternalOutput")

    with tile.TileContext(nc) as tc:
        cpool = tc.alloc_tile_pool(name="consts", bufs=1)
        wgpool = tc.alloc_tile_pool(name="wg", bufs=1)
        epool = tc.alloc_tile_pool(name="enc", bufs=1)
        ring = tc.alloc_tile_pool(name="ring", bufs=2)
        dpool = tc.alloc_tile_pool(name="dram", bufs=1, space="DRAM")
        encps = tc.alloc_tile_pool(name="encps", bufs=2, space="PSUM")
        bigps = tc.alloc_tile_pool(name="bigps", bufs=2, space="PSUM")

        # ---- index DMAs first on SP (they gate the gather chain) ----
        ctxi = cpool.tile([128, NT], I32)
        nc.sync.dma_start(out=ctxi[:], in_=d_ctx_idx[:, :])
        xi = cpool.tile([128, M], I32)
        nc.sync.dma_start(out=xi[:], in_=d_x_idx[:, :])

        # ---- W_gen fp8 streaming load on SP, group-ordered slices ----
        wg8 = wgpool.tile([128, N], FP8)
        c0 = 0
        for w in W_SLICES:
            c1 = min(c0 + w, N)
            if c1 > c0:
                nc.sync.dma_start(out=wg8[:, c0:c1], in_=d_wg8[:, c0:c1])
            c0 = c1

        # ---- small inputs issued from the ACT sequencer (idle early) ----
        identity_bf = cpool.tile([128, 128], BF16)
        nc.scalar.dma_start(out=identity_bf[:], in_=d_ident_bf[:, :])
        identity = cpool.tile([128, 128], F32)
        nc.scalar.dma_start(out=identity[:], in_=d_ident[:, :])
        waff1T = cpool.tile([128, 128], BF16)
        nc.scalar.dma_start(out=waff1T[:], in_=d_waff1T[:, :])
        waff2T = cpool.tile([128, 128], BF16)
        nc.scalar.dma_start(out=waff2T[:], in_=d_waff2T[:, :])
        wmuT = cpool.tile([128, 128], BF16)
        nc.scalar.dma_start(out=wmuT[:], in_=d_wmuT[:, :])
        wsigT = cpool.tile([128, 128], BF16)
        nc.scalar.dma_start(out=wsigT[:], in_=d_wsigT[:, :])
        baff = cpool.tile([128, 1], F32)
        nc.scalar.dma_start(out=baff[:], in_=d_baff[:, :])
        bmu = cpool.tile([128, 1], F32)
        nc.scalar.dma_start(out=bmu[:], in_=d_bmu[:, :])
        bsig = cpool.tile([128, 1], F32)
        nc.scalar.dma_start(out=bsig[:], in_=d_bsig[:, :])
        epsT = cpool.tile([128, BS], F32)
        nc.scalar.dma_start(out=epsT[:], in_=d_epsT[:, :])
        ctxbc = cpool.tile([128, M * C], I32)
        nc.scalar.dma_start(out=ctxbc[:], in_=d_ctx_byc[:, :])

        ones_col = cpool.tile([128, 1], F32)
        nc.vector.memset(ones_col, 1.0)
        if with_bgen:
            ones_bf = cpool.tile([1, 128], BF16)
            nc.vector.memset(ones_bf, 1.0)
        if SCH_SET:
            sch_c2 = cpool.tile([128, 1], F32)
            nc.vector.memset(sch_c2, SCH_C2)

        # ---- persistent tensors ----
        ctx_nat = epool.tile([128, TPM * 128 * M], BF16)  # gathered ctx rows
        cen_nat = epool.tile([128, 128 * M], BF16)        # center rows
        gsig_nat = epool.tile([128, 128 * M], F32)        # gsig rows
        ctxT = epool.tile([128, BS * C], BF16)
        centerT = epool.tile([128, BS], BF16)
        gsigT = epool.tile([128, BS], F32)
        h3 = epool.tile([128, BS * C], BF16)
        hsum_raw = epool.tile([128, BS], F32)
        hsumT = epool.tile([128, BS], BF16)
        napre = epool.tile([128, BS], F32)
        muT = epool.tile([128, BS], F32)
        infsigT = epool.tile([128, BS], F32)
        sp_tmp = epool.tile([128, 2 * BS], F32)
        sigmaT = epool.tile([128, BS], F32)
        lnsig = epool.tile([128, BS], F32)
        lninf = epool.tile([128, BS], F32)
        zT = epool.tile([128, BS], F32)
        z8 = epool.tile([128, BS], FP8)
        z_nat = epool.tile([128, M * (D + 1)], F32)
        wrows = epool.tile([128, M * C * (D + 1)], F32)
        tal_bc = epool.tile([128, M * C], F32)
        talsum = epool.tile([128, M], F32)
        sums = epool.tile([128, M * NG], F32)
        tot = epool.tile([128, M], F32)
        lse = epool.tile([128, M], F32)
        kli = epool.tile([128, BS], F32)
        knum = epool.tile([128, BS], F32)
        ktmp = epool.tile([128, BS], F32)
        kl_row = epool.tile([1, BS], F32)
        kl_rt = epool.tile([128, M], F32)
        loss_sb = epool.tile([128, M], F32)
        tal_prod = epool.tile([128, C * (D + 1)], F32)

        kl_d = dpool.tile([BS], F32)

        # ================= helpers =================
        def gather(dst, src_dram, idx_col):
            nc.gpsimd.indirect_dma_start(
                out=dst, out_offset=None, in_=src_dram[:, :],
                in_offset=IndirectOffsetOnAxis(ap=idx_col, axis=0))

        def transpose_copy(nat_cols, dst_cols, ident, dt=F32):
            ps = encps.tile([128, CHUNK], dt, tag="e", name="tps")
            nc.tensor.transpose(out=ps[:, :128], in_=nat_cols,
                                identity=ident[:, :])
            nc.vector.tensor_copy(out=dst_cols, in_=ps[:, :128])

        def emit_gathers(m):
            # center first (gates the apre path), then ctx tiles in order
            gather(cen_nat[:, m * 128:(m + 1) * 128], d_inf_bf, xi[:, m:m + 1])
            for t in range(TPM):
                a0 = (m * TPM + t) * 128
                gather(ctx_nat[:, a0:a0 + 128], d_inf_bf,
                       ctxi[:, m * TPM + t:m * TPM + t + 1])

        def emit_ctx_T(m, t):
            a0 = (m * TPM + t) * 128
            transpose_copy(ctx_nat[:, a0:a0 + 128], ctxT[:, a0:a0 + 128],
                           identity_bf, BF16)

        def emit_encoder(m):
            """Full per-m encoder; consumes ctx tiles per bpre chunk so it
            pipelines behind the gather train."""
            s0 = m * 128
            f0 = m * TPM * 128
            transpose_copy(cen_nat[:, s0:s0 + 128], centerT[:, s0:s0 + 128],
                           identity_bf, BF16)
            apre_ps = encps.tile([128, CHUNK], F32, tag="e", name="apre")
            nc.tensor.matmul(out=apre_ps[:, :128], lhsT=waff1T[:, :],
                             rhs=centerT[:, s0:s0 + 128], start=True, stop=True)
            # napre = -(apre_psum + baff)
            nc.vector.scalar_tensor_tensor(
                out=napre[:, s0:s0 + 128], in0=apre_ps[:, :128], scalar=-1.0,
                in1=baff[:, 0:1].to_broadcast([128, 128]),
                op0=ALU.mult, op1=ALU.subtract)
            done_t = 0
            for (j0, j1, tlo, thi) in BP_CHUNKS:
                for t in range(done_t, thi):
                    emit_ctx_T(m, t)
                done_t = thi
                nb = (j1 - j0) // C
                b0 = j0 // C
                bp = encps.tile([128, CHUNK], F32, tag="e", name="bp")
                nc.tensor.matmul(out=bp[:, :j1 - j0], lhsT=waff2T[:, :],
                                 rhs=ctxT[:, f0 + j0:f0 + j1],
                                 start=True, stop=True)
                # relu(a+b) = max(b,-a)+a
                nc.vector.tensor_tensor(
                    out=h3[:, f0 + j0:f0 + j1].rearrange("p (b c) -> p b c", c=C),
                    in0=bp[:, :j1 - j0].rearrange("p (b c) -> p b c", c=C),
                    in1=napre[:, s0 + b0:s0 + b0 + nb].to_broadcast([128, nb, C]),
                    op=ALU.max)
                nc.vector.reduce_sum(
                    out=hsum_raw[:, s0 + b0:s0 + b0 + nb],
                    in_=h3[:, f0 + j0:f0 + j1].rearrange("p (b c) -> p b c", c=C),
                    axis=AXX)
            # hsum = hsum_raw - C*napre  (= sum_c max + C*apre)
            nc.vector.scalar_tensor_tensor(
                out=hsumT[:, s0:s0 + 128], in0=napre[:, s0:s0 + 128],
                scalar=-float(C), in1=hsum_raw[:, s0:s0 + 128],
                op0=ALU.mult, op1=ALU.add)
            mu_ps = encps.tile([128, CHUNK], F32, tag="e", name="mups")
            nc.tensor.matmul(out=mu_ps[:, :128], lhsT=wmuT[:, :],
                             rhs=hsumT[:, s0:s0 + 128], start=True, stop=True)
            nc.vector.tensor_tensor(
                out=muT[:, s0:s0 + 128], in0=mu_ps[:, :128],
                in1=bmu[:, 0:1].to_broadcast([128, 128]), op=ALU.add)
            sig_ps = encps.tile([128, CHUNK], F32, tag="e", name="sigps")
            nc.tensor.matmul(out=sig_ps[:, :128], lhsT=wsigT[:, :],
                             rhs=hsumT[:, s0:s0 + 128], start=True, stop=True)
            # softplus = ln(1+exp(x+bsig))
            spc = sp_tmp[:, BS + s0:BS + s0 + 128]
            nc.scalar.activation(out=spc, in_=sig_ps[:, :128],
                                 func=AF.Exp, bias=bsig[:, 0:1], scale=1.0)
            nc.vector.tensor_scalar_add(out=spc, in0=spc, scalar1=1.0)
            nc.scalar.activation(out=infsigT[:, s0:s0 + 128], in_=spc, func=AF.Ln)
            # z = mu + eps * infsig
            nc.vector.tensor_tensor(out=zT[:, s0:s0 + 128],
                                    in0=epsT[:, s0:s0 + 128],
                                    in1=infsigT[:, s0:s0 + 128], op=ALU.mult)
            nc.vector.tensor_tensor(out=zT[:, s0:s0 + 128],
                                    in0=zT[:, s0:s0 + 128],
                                    in1=muT[:, s0:s0 + 128], op=ALU.add)
            nc.vector.tensor_scalar_mul(out=z8[:, s0:s0 + 128],
                                        in0=zT[:, s0:s0 + 128],
                                        scalar1=1.0 / WSCALE)
            # z natural layout + ones column (for exact take-along dots)
            zps = encps.tile([128, CHUNK], F32, tag="e", name="zps")
            nc.tensor.transpose(out=zps[:, :128], in_=zT[:, s0:s0 + 128],
                                identity=identity[:, :])
            a0 = m * (D + 1)
            nc.vector.tensor_copy(out=z_nat[:, a0:a0 + D], in_=zps[:, :128])
            nc.vector.tensor_copy(out=z_nat[:, a0 + D:a0 + D + 1],
                                  in_=ones_col[:, :])

        def emit_group(m, g):
            c0 = g * GROUP
            gw = min(GROUP, N - c0)
            ps = bigps.tile([128, GROUP], F32, tag="big", name="gps")
            for j0 in range(0, gw, CHUNK):
                j1 = min(j0 + CHUNK, gw)
                nc.tensor.matmul(out=ps[:, j0:j1],
                                 lhsT=z8[:, m * 128:(m + 1) * 128],
                                 rhs=wg8[:, c0 + j0:c0 + j1],
                                 start=True, stop=not with_bgen)
                if with_bgen:
                    bg = ring.tile([1, CHUNK], BF16, tag="bg", name="bg")
                    nc.sync.dma_start(
                        out=bg[:, :j1 - j0],
                        in_=d_bgen[(c0 + j0) // CHUNK, :j1 - j0]
                        .rearrange("(a b) -> a b", a=1))
                    nc.tensor.matmul(out=ps[:, j0:j1], lhsT=ones_bf[:, :],
                                     rhs=bg[:, :j1 - j0], start=False, stop=True)
            col = m * NG + g
            if (g % SCH_MOD) in SCH_SET and gw == GROUP:
                yi = ring.tile([128, GROUP], I16, tag="yi", name="yi")
                nc.vector.scalar_tensor_tensor(
                    out=yi[:, :gw], in0=ps[:, :gw], scalar=SCH_C1,
                    in1=sch_c2[:, 0:1].to_broadcast([128, gw]),
                    op0=ALU.mult, op1=ALU.add)
                nc.vector.tensor_reduce(out=sums[:, col:col + 1],
                                        in_=yi[:, :gw].bitcast(BF16),
                                        axis=AXX, op=ALU.add)
            else:
                eo = ring.tile([128, GROUP], BF16, tag="eo", name="eo")
                nc.scalar.activation(out=eo[:, :gw], in_=ps[:, :gw], func=AF.Exp,
                                     accum_out=sums[:, col:col + 1])

        def emit_gsig_gather(m):
            gather(gsig_nat[:, m * 128:(m + 1) * 128], d_gsig_emb,
                   xi[:, m:m + 1])

        def emit_gsig_T(m):
            transpose_copy(gsig_nat[:, m * 128:(m + 1) * 128],
                           gsigT[:, m * 128:(m + 1) * 128], identity)

        def emit_kl_act():
            nc.scalar.activation(out=sp_tmp[:, :BS], in_=gsigT[:, :], func=AF.Exp)
            nc.vector.tensor_scalar_add(out=sp_tmp[:, :BS], in0=sp_tmp[:, :BS],
                                        scalar1=1.0)
            nc.scalar.activation(out=sigmaT[:, :], in_=sp_tmp[:, :BS], func=AF.Ln)
            nc.scalar.activation(out=lnsig[:, :], in_=sigmaT[:, :], func=AF.Ln)
            nc.scalar.activation(out=lninf[:, :], in_=infsigT[:, :], func=AF.Ln)

        def emit_kl_dve():
            # kli = lnsig - lninf + (infsig^2 + (mu-sigma)^2)/(2 sigma^2) - 0.5
            nc.vector.tensor_tensor(out=kli[:, :], in0=lnsig[:, :],
                                    in1=lninf[:, :], op=ALU.subtract)
            nc.vector.tensor_tensor(out=knum[:, :], in0=muT[:, :],
                                    in1=sigmaT[:, :], op=ALU.subtract)
            nc.vector.tensor_tensor(out=knum[:, :], in0=knum[:, :],
                                    in1=knum[:, :], op=ALU.mult)
            nc.vector.tensor_tensor(out=ktmp[:, :], in0=infsigT[:, :],
                                    in1=infsigT[:, :], op=ALU.mult)
            nc.vector.tensor_tensor(out=knum[:, :], in0=knum[:, :],
                                    in1=ktmp[:, :], op=ALU.add)
            nc.vector.tensor_tensor(out=ktmp[:, :], in0=sigmaT[:, :],
                                    in1=sigmaT[:, :], op=ALU.mult)
            nc.vector.tensor_scalar_mul(out=ktmp[:, :], in0=ktmp[:, :],
                                        scalar1=2.0)
            nc.vector.reciprocal(out=ktmp[:, :], in_=ktmp[:, :])
            nc.vector.tensor_tensor(out=knum[:, :], in0=knum[:, :],
                                    in1=ktmp[:, :], op=ALU.mult)
            nc.vector.tensor_tensor(out=kli[:, :], in0=kli[:, :],
                                    in1=knum[:, :], op=ALU.add)
            nc.vector.tensor_scalar_add(out=kli[:, :], in0=kli[:, :],
                                        scalar1=-0.5)

        def emit_kl_reduce():
            kl_ps = encps.tile([1, CHUNK], F32, tag="e", name="klps")
            nc.tensor.matmul(out=kl_ps[:1, :BS], lhsT=ones_col[:, :],
                             rhs=kli[:, :], start=True, stop=True)
            nc.vector.tensor_copy(out=kl_row[:, :], in_=kl_ps[:1, :BS])
            nc.sync.dma_start(out=kl_d[:].rearrange("(a b) -> a b", a=1),
                              in_=kl_row[:, :])
            nc.sync.dma_start(out=kl_rt[:, :],
                              in_=kl_d[:].rearrange("(m p) -> p m", p=128))

        def emit_wrows_gathers():
            for t in range(M * C):
                gather(wrows[:, t * (D + 1):(t + 1) * (D + 1)], d_wg_aug,
                       ctxbc[:, t:t + 1])

        def emit_tal():
            # exact take-along dots, batched: one mult + one reduce per m
            DP = D + 1
            for m in range(M):
                a0 = m * DP
                w0 = m * C * DP
                zb = (z_nat[:, a0:a0 + DP]
                      .rearrange("p (a d) -> p a d", a=1)
                      .to_broadcast([128, C, DP]))
                nc.vector.tensor_tensor(
                    out=tal_prod[:, :].rearrange("p (c d) -> p c d", d=DP),
                    in0=wrows[:, w0:w0 + C * DP]
                    .rearrange("p (c d) -> p c d", d=DP),
                    in1=zb, op=ALU.mult)
                nc.vector.reduce_sum(
                    out=tal_bc[:, m * C:(m + 1) * C],
                    in_=tal_prod[:, :].rearrange("p (c d) -> p c d", d=DP),
                    axis=AXX)
            for m in range(M):
                nc.vector.reduce_sum(out=talsum[:, m:m + 1],
                                     in_=tal_bc[:, m * C:(m + 1) * C], axis=AXX)

        def emit_epilogue():
            for m in range(M):
                nc.vector.reduce_sum(out=tot[:, m:m + 1],
                                     in_=sums[:, m * NG:(m + 1) * NG], axis=AXX)
            nc.scalar.activation(out=lse[:, :], in_=tot[:, :], func=AF.Ln)
            nc.vector.tensor_tensor(out=loss_sb[:, :], in0=kl_rt[:, :],
                                    in1=talsum[:, :], op=ALU.subtract)
            nc.vector.scalar_tensor_tensor(
                out=loss_sb[:, :], in0=lse[:, :], scalar=float(C),
                in1=loss_sb[:, :], op0=ALU.mult, op1=ALU.add)
            nc.sync.dma_start(out=d_loss[:].rearrange("(m p) -> p m", p=128),
                              in_=loss_sb[:, :])

        # ================= schedule =================
        # GpSimd stream: m0 gathers -> m1 gathers -> gsig -> wrows
        emit_gathers(0)
        emit_gathers(1)
        emit_gsig_gather(0)
        emit_gsig_gather(1)
        emit_wrows_gathers()

        # m0 encoder (pipelines behind the m0 gather train)
        emit_encoder(0)

        # m0 streaming train; m1 encoder as one block mid-train (its gathers
        # complete ~40us in)
        for g in range(NG):
            emit_group(0, g)
            if g == 14:
                emit_encoder(1)
                emit_gsig_T(0)
                emit_gsig_T(1)

        # between trains: KL math (all encoder outputs ready)
        emit_kl_act()
        emit_kl_dve()

        # m1 streaming train; kl partition-reduce + roundtrip injected at g4,
        # take-along dots (DVE, need wrows gathers done ~63us) at g8
        for g in range(NG):
            emit_group(1, g)
            if g == 4:
                emit_kl_reduce()
            elif g == 8:
                emit_tal()

        emit_epilogue()

        bigps.release()
        encps.release()
        dpool.release()
        ring.release()
        epool.release()
        wgpool.release()
        cpool.release()

    nc.compile()
    return nc


def _prep_inputs(x_batch, context_words_batch, eps, inf_emb, W_aff, b_aff,
                 W_mu, b_mu, W_sig, b_sig, gen_sigma_emb, W_gen, b_gen,
                 with_bgen):
    f32 = lambda a: np.ascontiguousarray(np.asarray(a, dtype=np.float32))
    bf16 = lambda a: np.ascontiguousarray(
        np.asarray(a, dtype=np.float32).astype(ml_dtypes.bfloat16))
    x_batch = np.asarray(x_batch, dtype=np.int32)
    ctx = np.asarray(context_words_batch, dtype=np.int32)
    eps = f32(eps)
    W_aff = np.asarray(W_aff, dtype=np.float32)
    inf_emb, gen_sigma_emb = f32(inf_emb), f32(gen_sigma_emb)
    W_gen = np.asarray(W_gen, dtype=np.float32)
    b_gen = np.asarray(b_gen, dtype=np.float32)

    shared = {
        "waff1T": bf16(W_aff[:, :D].T),
        "waff2T": bf16(W_aff[:, D:].T),
        "wmuT": bf16(np.asarray(W_mu, dtype=np.float32).T),
        "wsigT": bf16(np.asarray(W_sig, dtype=np.float32).T),
        "baff": f32(np.asarray(b_aff).reshape(D, 1)),
        "bmu": f32(np.asarray(b_mu).reshape(D, 1)),
        "bsig": f32(np.asarray(b_sig).reshape(D, 1)),
        "inf_bf": bf16(inf_emb),
        "gsig_emb": gen_sigma_emb,
        "wg_aug": np.ascontiguousarray(
            np.concatenate([W_gen, b_gen.reshape(N, 1)], axis=1)),
        "wg8": np.ascontiguousarray(
            (W_gen.T * WSCALE).astype(ml_dtypes.float8_e4m3)),
        "ident": np.eye(128, dtype=np.float32),
        "ident_bf": np.eye(128, dtype=ml_dtypes.bfloat16),
    }
    if with_bgen:
        NCH = (N + CHUNK - 1) // CHUNK
        bg = np.zeros((NCH * CHUNK,), dtype=ml_dtypes.bfloat16)
        bg[:N] = b_gen.astype(ml_dtypes.bfloat16)
        shared["bgen2d"] = bg.reshape(NCH, CHUNK)

    in_maps = []
    for s in range(NCORES):
        lo, hi = s * BS, (s + 1) * BS
        csh = ctx[lo:hi]                      # [BS, C]
        m = dict(shared)
        m["ctx_idx"] = np.ascontiguousarray(
            csh.reshape(BS * C).reshape(NT, 128).T)
        m["ctx_byc"] = np.ascontiguousarray(
            np.concatenate([csh[k * 128:(k + 1) * 128, :] for k in range(M)],
                           axis=1))
        m["x_idx"] = np.ascontiguousarray(x_batch[lo:hi].reshape(M, 128).T)
        m["epsT"] = np.ascontiguousarray(eps[lo:hi].T)
        in_maps.append(m)
    return in_maps


def kernel(x_batch, context_words_batch, eps, inf_emb, W_aff, b_aff,
           W_mu, b_mu, W_sig, b_sig, gen_sigma_emb, W_gen, b_gen,
           trace=False):
    with_bgen = bool(np.any(np.asarray(b_gen) != 0))
    if with_bgen not in _CACHE:
        _CACHE[with_bgen] = _build(with_bgen)
    nc = _CACHE[with_bgen]

    in_maps = _prep_inputs(x_batch, context_words_batch, eps, inf_emb, W_aff,
                           b_aff, W_mu, b_mu, W_sig, b_sig, gen_sigma_emb,
                           W_gen, b_gen, with_bgen)
    res = run_bass_kernel_spmd(nc, in_maps, core_ids=list(range(NCORES)),
                               trace=trace)
    parts = [res.results[s]["loss_part"] for s in range(NCORES)]
    loss = np.concatenate(parts).astype(np.float64).mean()
    out = np.float32(loss)
    if trace:
        kernel.last_results = res
    return out


# revision 15
# speedup vs baseline: 1.2208x; 1.0196x over previous
"""Trainium2 Bass kernel for the BayesianSkipgram loss.

Strategy (8 NeuronCores, batch-sharded, no collectives):
  - Each core computes the per-sample loss for its 256-sample batch shard.
  - The dominant cost is sum(exp(logits)) over the 50257-vocab. The ACT
    engine's Exp (0.833ns/elem, with free per-group accumulation) and a
    DVE Schraudolph bit-trick exp (scalar_tensor_tensor -> int16, bitcast
    bf16, reduce) split the vocab groups ~73/27 so both engines stream
    concurrently. Everything else hides under the two 33-group trains.
  - Pipelined start: chunk m0's center gather lands first, the encoder
    consumes ctx tiles per bpre-chunk as gathers land, so the m0 train
    starts as early as possible. m1's encoder runs as one block in the
    middle of the m0 train; KL and take-along overlap the trains.
  - W_gen^T is pre-scaled x8, cast to fp8e4 on host (6.4MB), streamed in
    28 column-slices so group 0 lands before the first matmul. z is scaled
    1/8 to fp8 for the logits matmul (PSUM accumulates f32); logit rms
    error ~4% of logit sigma, negligible for logsumexp at 2e-2 tolerance.
  - PSUM: 6 banks = 2 x [128,1536] streaming tiles, 2 banks = encoder ring.
  - take_along(logp) is exact: fp32 indirect-DMA row gathers of
    [W_gen | b_gen] dotted against fp32 z on the (otherwise idle) GpSimd.
  - Host combines the 8x[256] per-sample losses with a mean.
"""

import numpy as np
import ml_dtypes

import concourse.bass as bass
import concourse.mybir as mybir
import concourse.tile as tile
from concourse import bacc
from concourse.bass import IndirectOffsetOnAxis
from concourse.bass_utils import run_bass_kernel_spmd

F32 = mybir.dt.float32
BF16 = mybir.dt.bfloat16
FP8 = mybir.dt.float8e4
I16 = mybir.dt.int16
I32 = mybir.dt.int32
AF = mybir.ActivationFunctionType
ALU = mybir.AluOpType
AXX = mybir.AxisListType.X

N = 50257      # vocab
D = 128        # embedding dim
B = 2048       # total batch
C = 10         # context size
NCORES = 8
BS = B // NCORES          # 256 samples per core
M = BS // 128             # 2 partition chunks of samples
NT = BS * C // 128        # 20 ctx gather tiles (10 per m)
TPM = NT // M             # 10
GROUP = 1536              # streaming group (3 psum banks)
NG = (N + GROUP - 1) // GROUP        # 33 groups
CHUNK = 512               # matmul free-dim
WSCALE = 8.0              # W_gen pre-scale (z scaled 1/WSCALE)

# Schraudolph DVE-exp offload: groups with g % SCH_MOD in SCH_SET go to the
# DVE. Empty set = all exact ACT exp.
SCH_MOD = 7
SCH_SET = frozenset({2, 5})
SCH_C1 = 184.6650390625   # 2^7 * log2(e)
SCH_C2 = 16250.25         # calibrated: 127*2^7 minus mean-centering tweak

# bpre is computed in C-aligned chunks; chunk k consumes ctx tiles [lo, hi)
BCH = 51 * C              # 510 cols = 51 samples
BP_CHUNKS = [(0, 510, 0, 4), (510, 1020, 3, 8), (1020, 1280, 7, 10)]

# W load slicing: first fine slices for fast group-0 arrival, then coarse.
W_SLICES = [1024] * 6 + [2048] * 22   # 6*1024 + 22*2048 = 51200 >= N

_CACHE = {}


def _patch_act_tables():
    """Keep Exp/Ln/Identity/Copy only in natural_log_exp_and_others so the
    table-load inserter uses one set for the whole kernel."""
    import concourse.bacc as _bacc_mod
    import concourse.hw_specs as _hws
    if getattr(_bacc_mod, "_ant_act_tables_patched", False):
        return
    _orig = _hws.get_activation_tables
    _ours = {AF.Exp, AF.Ln, AF.Identity, AF.Copy}

    def _filtered(arch):
        tabs = _orig(arch)
        out = {}
        for name, funcs in tabs.items():
            if name == "natural_log_exp_and_others" or not (_ours & funcs):
                out[name] = funcs
            else:
                out[name] = funcs - _ours
        return out

    _bacc_mod.get_activation_tables = _filtered
    _bacc_mod._ant_act_tables_patched = True


def _build(with_bgen: bool):
    _patch_act_tables()
    nc = bacc.Bacc("TRN2", target_bir_lowering=False, debug=False)

    # ---------------- DRAM I/O ----------------
    d_ctx_idx = nc.dram_tensor("ctx_idx", [128, NT], I32, kind="ExternalInput")
    d_ctx_byc = nc.dram_tensor("ctx_byc", [128, M * C], I32, kind="ExternalInput")
    d_x_idx = nc.dram_tensor("x_idx", [128, M], I32, kind="ExternalInput")
    d_epsT = nc.dram_tensor("epsT", [128, BS], F32, kind="ExternalInput")
    d_waff1T = nc.dram_tensor("waff1T", [128, 128], BF16, kind="ExternalInput")
    d_waff2T = nc.dram_tensor("waff2T", [128, 128], BF16, kind="ExternalInput")
    d_wmuT = nc.dram_tensor("wmuT", [128, 128], BF16, kind="ExternalInput")
    d_wsigT = nc.dram_tensor("wsigT", [128, 128], BF16, kind="ExternalInput")
    d_baff = nc.dram_tensor("baff", [128, 1], F32, kind="ExternalInput")
    d_bmu = nc.dram_tensor("bmu", [128, 1], F32, kind="ExternalInput")
    d_bsig = nc.dram_tensor("bsig", [128, 1], F32, kind="ExternalInput")
    d_inf_bf = nc.dram_tensor("inf_bf", [N, D], BF16, kind="ExternalInput")
    d_gsig_bf = nc.dram_tensor("gsig_bf", [N, D], BF16, kind="ExternalInput")
    d_wg_aug = nc.dram_tensor("wg_aug", [N, D + 2], BF16, kind="ExternalInput")
    d_wg8 = nc.dram_tensor("wg8", [128, N], FP8, kind="ExternalInput")
    d_ident = nc.dram_tensor("ident", [128, 128], F32, kind="ExternalInput")
    d_ident_bf = nc.dram_tensor("ident_bf", [128, 128], BF16, kind="ExternalInput")
    if with_bgen:
        NCH = (N + CHUNK - 1) // CHUNK
        d_bgen = nc.dram_tensor("bgen2d", [NCH, CHUNK], BF16, kind="ExternalInput")
    d_loss = nc.dram_tensor("loss_part", [BS], F32, kind="ExternalOutput")

    with tile.TileContext(nc) as tc:
        cpool = tc.alloc_tile_pool(name="consts", bufs=1)
        wgpool = tc.alloc_tile_pool(name="wg", bufs=1)
        epool = tc.alloc_tile_pool(name="enc", bufs=1)
        ring = tc.alloc_tile_pool(name="ring", bufs=2)
        dpool = tc.alloc_tile_pool(name="dram", bufs=1, space="DRAM")
        encps = tc.alloc_tile_pool(name="encps", bufs=2, space="PSUM")
        bigps = tc.alloc_tile_pool(name="bigps", bufs=2, space="PSUM")

        # ---- index DMAs first on SP (they gate the gather chain) ----
        ctxi = cpool.tile([128, NT], I32)
        nc.sync.dma_start(out=ctxi[:], in_=d_ctx_idx[:, :])
        xi = cpool.tile([128, M], I32)
        nc.sync.dma_start(out=xi[:], in_=d_x_idx[:, :])

        # ---- W_gen fp8 streaming load on SP, group-ordered slices ----
        wg8 = wgpool.tile([128, N], FP8)
        c0 = 0
        for w in W_SLICES:
            c1 = min(c0 + w, N)
            if c1 > c0:
                nc.sync.dma_start(out=wg8[:, c0:c1], in_=d_wg8[:, c0:c1])
            c0 = c1

        # ---- small inputs issued from the ACT sequencer (idle early) ----
        identity_bf = cpool.tile([128, 128], BF16)
        nc.scalar.dma_start(out=identity_bf[:], in_=d_ident_bf[:, :])
        identity = cpool.tile([128, 128], F32)
        nc.scalar.dma_start(out=identity[:], in_=d_ident[:, :])
        waff1T = cpool.tile([128, 128], BF16)
        nc.scalar.dma_start(out=waff1T[:], in_=d_waff1T[:, :])
        waff2T = cpool.tile([128, 128], BF16)
        nc.scalar.dma_start(out=waff2T[:], in_=d_waff2T[:, :])
        wmuT = cpool.tile([128, 128], BF16)
        nc.scalar.dma_start(out=wmuT[:], in_=d_wmuT[:, :])
        wsigT = cpool.tile([128, 128], BF16)
        nc.scalar.dma_start(out=wsigT[:], in_=d_wsigT[:, :])
        baff = cpool.tile([128, 1], F32)
        nc.scalar.dma_start(out=baff[:], in_=d_baff[:, :])
        bmu = cpool.tile([128, 1], F32)
        nc.scalar.dma_start(out=bmu[:], in_=d_bmu[:, :])
        bsig = cpool.tile([128, 1], F32)
        nc.scalar.dma_start(out=bsig[:], in_=d_bsig[:, :])
        epsT = cpool.tile([128, BS], F32)
        nc.scalar.dma_start(out=epsT[:], in_=d_epsT[:, :])
        ctxbc = cpool.tile([128, M * C], I32)
        nc.scalar.dma_start(out=ctxbc[:], in_=d_ctx_byc[:, :])

        ones_col = cpool.tile([128, 1], F32)
        nc.vector.memset(ones_col, 1.0)
        if with_bgen:
            ones_bf = cpool.tile([1, 128], BF16)
            nc.vector.memset(ones_bf, 1.0)
        if SCH_SET:
            sch_c2 = cpool.tile([128, 1], F32)
            nc.vector.memset(sch_c2, SCH_C2)

        # ---- persistent tensors ----
        ctx_nat = epool.tile([128, TPM * 128 * M], BF16)  # gathered ctx rows
        cen_nat = epool.tile([128, 128 * M], BF16)        # center rows
        gsig_nat = epool.tile([128, 128 * M], BF16)       # gsig rows
        ctxT = epool.tile([128, BS * C], BF16)
        centerT = epool.tile([128, BS], BF16)
        gsigT = epool.tile([128, BS], F32)
        h3 = epool.tile([128, BS * C], BF16)
        hsum_raw = epool.tile([128, BS], F32)
        hsumT = epool.tile([128, BS], BF16)
        napre = epool.tile([128, BS], F32)
        muT = epool.tile([128, BS], F32)
        infsigT = epool.tile([128, BS], F32)
        sp_tmp = epool.tile([128, 2 * BS], F32)
        sigmaT = epool.tile([128, BS], F32)
        lnsig = epool.tile([128, BS], F32)
        lninf = epool.tile([128, BS], F32)
        zT = epool.tile([128, BS], F32)
        z8 = epool.tile([128, BS], FP8)
        z_nat = epool.tile([128, M * (D + 2)], F32)
        wrows = epool.tile([128, M * C * (D + 2)], BF16)
        tal_bc = epool.tile([128, M * C], F32)
        talsum = epool.tile([128, M], F32)
        sums = epool.tile([128, M * NG], F32)
        tot = epool.tile([128, M], F32)
        lse = epool.tile([128, M], F32)
        kli = epool.tile([128, BS], F32)
        knum = epool.tile([128, BS], F32)
        ktmp = epool.tile([128, BS], F32)
        kl_row = epool.tile([1, BS], F32)
        kl_rt = epool.tile([128, M], F32)
        loss_sb = epool.tile([128, M], F32)
        tal_prod = epool.tile([128, D + 2], F32)

        kl_d = dpool.tile([BS], F32)

        # ================= helpers =================
        def gather(dst, src_dram, idx_col):
            nc.gpsimd.indirect_dma_start(
                out=dst, out_offset=None, in_=src_dram[:, :],
                in_offset=IndirectOffsetOnAxis(ap=idx_col, axis=0))

        def transpose_copy(nat_cols, dst_cols, ident, dt=F32):
            ps = encps.tile([128, CHUNK], dt, tag="e", name="tps")
            nc.tensor.transpose(out=ps[:, :128], in_=nat_cols,
                                identity=ident[:, :])
            nc.vector.tensor_copy(out=dst_cols, in_=ps[:, :128])

        def emit_gathers(m):
            # center first (gates the apre path), then ctx tiles in order
            gather(cen_nat[:, m * 128:(m + 1) * 128], d_inf_bf, xi[:, m:m + 1])
            for t in range(TPM):
                a0 = (m * TPM + t) * 128
                gather(ctx_nat[:, a0:a0 + 128], d_inf_bf,
                       ctxi[:, m * TPM + t:m * TPM + t + 1])

        def emit_ctx_T(m, t):
            a0 = (m * TPM + t) * 128
            transpose_copy(ctx_nat[:, a0:a0 + 128], ctxT[:, a0:a0 + 128],
                           identity_bf, BF16)

        def emit_encoder(m):
            """Full per-m encoder; consumes ctx tiles per bpre chunk so it
            pipelines behind the gather train."""
            s0 = m * 128
            f0 = m * TPM * 128
            transpose_copy(cen_nat[:, s0:s0 + 128], centerT[:, s0:s0 + 128],
                           identity_bf, BF16)
            apre_ps = encps.tile([128, CHUNK], F32, tag="e", name="apre")
            nc.tensor.matmul(out=apre_ps[:, :128], lhsT=waff1T[:, :],
                             rhs=centerT[:, s0:s0 + 128], start=True, stop=True)
            # napre = -(apre_psum + baff)
            nc.vector.scalar_tensor_tensor(
                out=napre[:, s0:s0 + 128], in0=apre_ps[:, :128], scalar=-1.0,
                in1=baff[:, 0:1].to_broadcast([128, 128]),
                op0=ALU.mult, op1=ALU.subtract)
            done_t = 0
            for (j0, j1, tlo, thi) in BP_CHUNKS:
                for t in range(done_t, thi):
                    emit_ctx_T(m, t)
                done_t = thi
                nb = (j1 - j0) // C
                b0 = j0 // C
                bp = encps.tile([128, CHUNK], F32, tag="e", name="bp")
                nc.tensor.matmul(out=bp[:, :j1 - j0], lhsT=waff2T[:, :],
                                 rhs=ctxT[:, f0 + j0:f0 + j1],
                                 start=True, stop=True)
                # relu(a+b) = max(b,-a)+a
                nc.vector.tensor_tensor(
                    out=h3[:, f0 + j0:f0 + j1].rearrange("p (b c) -> p b c", c=C),
                    in0=bp[:, :j1 - j0].rearrange("p (b c) -> p b c", c=C),
                    in1=napre[:, s0 + b0:s0 + b0 + nb].to_broadcast([128, nb, C]),
                    op=ALU.max)
                nc.vector.reduce_sum(
                    out=hsum_raw[:, s0 + b0:s0 + b0 + nb],
                    in_=h3[:, f0 + j0:f0 + j1].rearrange("p (b c) -> p b c", c=C),
                    axis=AXX)
            # hsum = hsum_raw - C*napre  (= sum_c max + C*apre)
            nc.vector.scalar_tensor_tensor(
                out=hsumT[:, s0:s0 + 128], in0=napre[:, s0:s0 + 128],
                scalar=-float(C), in1=hsum_raw[:, s0:s0 + 128],
                op0=ALU.mult, op1=ALU.add)
            mu_ps = encps.tile([128, CHUNK], F32, tag="e", name="mups")
            nc.tensor.matmul(out=mu_ps[:, :128], lhsT=wmuT[:, :],
                             rhs=hsumT[:, s0:s0 + 128], start=True, stop=True)
            nc.vector.tensor_tensor(
                out=muT[:, s0:s0 + 128], in0=mu_ps[:, :128],
                in1=bmu[:, 0:1].to_broadcast([128, 128]), op=ALU.add)
            sig_ps = encps.tile([128, CHUNK], F32, tag="e", name="sigps")
            nc.tensor.matmul(out=sig_ps[:, :128], lhsT=wsigT[:, :],
                             rhs=hsumT[:, s0:s0 + 128], start=True, stop=True)
            # softplus = ln(1+exp(x+bsig))
            spc = sp_tmp[:, BS + s0:BS + s0 + 128]
            nc.scalar.activation(out=spc, in_=sig_ps[:, :128],
                                 func=AF.Exp, bias=bsig[:, 0:1], scale=1.0)
            nc.vector.tensor_scalar_add(out=spc, in0=spc, scalar1=1.0)
            nc.scalar.activation(out=infsigT[:, s0:s0 + 128], in_=spc, func=AF.Ln)
            # z = mu + eps * infsig
            nc.vector.tensor_tensor(out=zT[:, s0:s0 + 128],
                                    in0=epsT[:, s0:s0 + 128],
                                    in1=infsigT[:, s0:s0 + 128], op=ALU.mult)
            nc.vector.tensor_tensor(out=zT[:, s0:s0 + 128],
                                    in0=zT[:, s0:s0 + 128],
                                    in1=muT[:, s0:s0 + 128], op=ALU.add)
            nc.vector.tensor_scalar_mul(out=z8[:, s0:s0 + 128],
                                        in0=zT[:, s0:s0 + 128],
                                        scalar1=1.0 / WSCALE)
            # z natural layout + ones column (for exact take-along dots)
            zps = encps.tile([128, CHUNK], F32, tag="e", name="zps")
            nc.tensor.transpose(out=zps[:, :128], in_=zT[:, s0:s0 + 128],
                                identity=identity[:, :])
            a0 = m * (D + 2)
            nc.vector.tensor_copy(out=z_nat[:, a0:a0 + D], in_=zps[:, :128])
            nc.vector.tensor_copy(out=z_nat[:, a0 + D:a0 + D + 1],
                                  in_=ones_col[:, :])
            nc.vector.memset(z_nat[:, a0 + D + 1:a0 + D + 2], 0.0)

        def emit_group(m, g):
            c0 = g * GROUP
            gw = min(GROUP, N - c0)
            ps = bigps.tile([128, GROUP], F32, tag="big", name="gps")
            for j0 in range(0, gw, CHUNK):
                j1 = min(j0 + CHUNK, gw)
                nc.tensor.matmul(out=ps[:, j0:j1],
                                 lhsT=z8[:, m * 128:(m + 1) * 128],
                                 rhs=wg8[:, c0 + j0:c0 + j1],
                                 start=True, stop=not with_bgen)
                if with_bgen:
                    bg = ring.tile([1, CHUNK], BF16, tag="bg", name="bg")
                    nc.sync.dma_start(
                        out=bg[:, :j1 - j0],
                        in_=d_bgen[(c0 + j0) // CHUNK, :j1 - j0]
                        .rearrange("(a b) -> a b", a=1))
                    nc.tensor.matmul(out=ps[:, j0:j1], lhsT=ones_bf[:, :],
                                     rhs=bg[:, :j1 - j0], start=False, stop=True)
            col = m * NG + g
            if (g % SCH_MOD) in SCH_SET and gw == GROUP:
                yi = ring.tile([128, GROUP], I16, tag="yi", name="yi")
                nc.vector.scalar_tensor_tensor(
                    out=yi[:, :gw], in0=ps[:, :gw], scalar=SCH_C1,
                    in1=sch_c2[:, 0:1].to_broadcast([128, gw]),
                    op0=ALU.mult, op1=ALU.add)
                nc.vector.tensor_reduce(out=sums[:, col:col + 1],
                                        in_=yi[:, :gw].bitcast(BF16),
                                        axis=AXX, op=ALU.add)
            else:
                eo = ring.tile([128, GROUP], BF16, tag="eo", name="eo")
                nc.scalar.activation(out=eo[:, :gw], in_=ps[:, :gw], func=AF.Exp,
                                     accum_out=sums[:, col:col + 1])

        def emit_gsig_gather(m):
            gather(gsig_nat[:, m * 128:(m + 1) * 128], d_gsig_bf,
                   xi[:, m:m + 1])

        def emit_gsig_T(m):
            transpose_copy(gsig_nat[:, m * 128:(m + 1) * 128],
                           gsigT[:, m * 128:(m + 1) * 128], identity_bf, BF16)

        def emit_kl_act():
            nc.scalar.activation(out=sp_tmp[:, :BS], in_=gsigT[:, :], func=AF.Exp)
            nc.vector.tensor_scalar_add(out=sp_tmp[:, :BS], in0=sp_tmp[:, :BS],
                                        scalar1=1.0)
            nc.scalar.activation(out=sigmaT[:, :], in_=sp_tmp[:, :BS], func=AF.Ln)
            nc.scalar.activation(out=lnsig[:, :], in_=sigmaT[:, :], func=AF.Ln)
            nc.scalar.activation(out=lninf[:, :], in_=infsigT[:, :], func=AF.Ln)

        def emit_kl_dve():
            # kli = lnsig - lninf + (infsig^2 + (mu-sigma)^2)/(2 sigma^2) - 0.5
            nc.vector.tensor_tensor(out=kli[:, :], in0=lnsig[:, :],
                                    in1=lninf[:, :], op=ALU.subtract)
            nc.vector.tensor_tensor(out=knum[:, :], in0=muT[:, :],
                                    in1=sigmaT[:, :], op=ALU.subtract)
            nc.vector.tensor_tensor(out=knum[:, :], in0=knum[:, :],
                                    in1=knum[:, :], op=ALU.mult)
            nc.vector.tensor_tensor(out=ktmp[:, :], in0=infsigT[:, :],
                                    in1=infsigT[:, :], op=ALU.mult)
            nc.vector.tensor_tensor(out=knum[:, :], in0=knum[:, :],
                                    in1=ktmp[:, :], op=ALU.add)
            nc.vector.tensor_tensor(out=ktmp[:, :], in0=sigmaT[:, :],
                                    in1=sigmaT[:, :], op=ALU.mult)
            nc.vector.tensor_scalar_mul(out=ktmp[:, :], in0=ktmp[:, :],
                                        scalar1=2.0)
            nc.vector.reciprocal(out=ktmp[:, :], in_=ktmp[:, :])
            nc.vector.tensor_tensor(out=knum[:, :], in0=knum[:, :],
                                    in1=ktmp[:, :], op=ALU.mult)
            nc.vector.tensor_tensor(out=kli[:, :], in0=kli[:, :],
                                    in1=knum[:, :], op=ALU.add)
            nc.vector.tensor_scalar_add(out=kli[:, :], in0=kli[:, :],
                                        scalar1=-0.5)

        def emit_kl_reduce():
            kl_ps = encps.tile([1, CHUNK], F32, tag="e", name="klps")
            nc.tensor.matmul(out=kl_ps[:1, :BS], lhsT=ones_col[:, :],
                             rhs=kli[:, :], start=True, stop=True)
            nc.vector.tensor_copy(out=kl_row[:, :], in_=kl_ps[:1, :BS])
            nc.sync.dma_start(out=kl_d[:].rearrange("(a b) -> a b", a=1),
                              in_=kl_row[:, :])
            nc.sync.dma_start(out=kl_rt[:, :],
                              in_=kl_d[:].rearrange("(m p) -> p m", p=128))

        def emit_wrows_gathers():
            for t in range(M * C):
                gather(wrows[:, t * (D + 2):(t + 1) * (D + 2)], d_wg_aug,
                       ctxbc[:, t:t + 1])

        def emit_tal():
            # exact take-along dots; one (mult, reduce) pair per (m, c) so
            # each op depends on exactly one wrows gather
            DP = D + 2
            for m in range(M):
                a0 = m * DP
                for c in range(C):
                    t = m * C + c
                    w0 = t * DP
                    nc.vector.tensor_tensor(
                        out=tal_prod[:, :], in0=wrows[:, w0:w0 + DP],
                        in1=z_nat[:, a0:a0 + DP], op=ALU.mult)
                    nc.vector.reduce_sum(out=tal_bc[:, t:t + 1],
                                         in_=tal_prod[:, :], axis=AXX)
            for m in range(M):
                nc.vector.reduce_sum(out=talsum[:, m:m + 1],
                                     in_=tal_bc[:, m * C:(m + 1) * C], axis=AXX)

        def emit_epilogue():
            for m in range(M):
                nc.vector.reduce_sum(out=tot[:, m:m + 1],
                                     in_=sums[:, m * NG:(m + 1) * NG], axis=AXX)
            nc.scalar.activation(out=lse[:, :], in_=tot[:, :], func=AF.Ln)
            nc.vector.tensor_tensor(out=loss_sb[:, :], in0=kl_rt[:, :],
                                    in1=talsum[:, :], op=ALU.subtract)
            nc.vector.scalar_tensor_tensor(
                out=loss_sb[:, :], in0=lse[:, :], scalar=float(C),
                in1=loss_sb[:, :], op0=ALU.mult, op1=ALU.add)
            nc.sync.dma_start(out=d_loss[:].rearrange("(m p) -> p m", p=128),
                              in_=loss_sb[:, :])

        # ================= schedule =================
        # GpSimd stream: m0 gathers -> m1 gathers -> gsig -> wrows
        emit_gathers(0)
        emit_gathers(1)
        emit_gsig_gather(0)
        emit_gsig_gather(1)
        emit_wrows_gathers()

        # m0 encoder (pipelines behind the m0 gather train)
        emit_encoder(0)

        # m0 streaming train; m1 encoder as one block mid-train (its gathers
        # complete ~40us in)
        for g in range(NG):
            emit_group(0, g)
            if g == 14:
                emit_encoder(1)
                emit_gsig_T(0)
                emit_gsig_T(1)

        # between trains: KL math (all encoder outputs ready)
        emit_kl_act()
        emit_kl_dve()

        # m1 streaming train; kl partition-reduce + roundtrip injected at g4,
        # take-along dots (DVE, need wrows gathers done ~63us) at g8
        for g in range(NG):
            emit_group(1, g)
            if g == 4:
                emit_kl_reduce()

        emit_tal()
        emit_epilogue()

        bigps.release()
        encps.release()
        dpool.release()
        ring.release()
        epool.release()
        wgpool.release()
        cpool.release()

    nc.compile()
    return nc


def _prep_inputs(x_batch, context_words_batch, eps, inf_emb, W_aff, b_aff,
                 W_mu, b_mu, W_sig, b_sig, gen_sigma_emb, W_gen, b_gen,
                 with_bgen):
    f32 = lambda a: np.ascontiguousarray(np.asarray(a, dtype=np.float32))
    bf16 = lambda a: np.ascontiguousarray(
        np.asarray(a, dtype=np.float32).astype(ml_dtypes.bfloat16))
    x_batch = np.asarray(x_batch, dtype=np.int32)
    ctx = np.asarray(context_words_batch, dtype=np.int32)
    eps = f32(eps)
    W_aff = np.asarray(W_aff, dtype=np.float32)
    inf_emb, gen_sigma_emb = f32(inf_emb), f32(gen_sigma_emb)
    W_gen = np.asarray(W_gen, dtype=np.float32)
    b_gen = np.asarray(b_gen, dtype=np.float32)

    shared = {
        "waff1T": bf16(W_aff[:, :D].T),
        "waff2T": bf16(W_aff[:, D:].T),
        "wmuT": bf16(np.asarray(W_mu, dtype=np.float32).T),
        "wsigT": bf16(np.asarray(W_sig, dtype=np.float32).T),
        "baff": f32(np.asarray(b_aff).reshape(D, 1)),
        "bmu": f32(np.asarray(b_mu).reshape(D, 1)),
        "bsig": f32(np.asarray(b_sig).reshape(D, 1)),
        "inf_bf": bf16(inf_emb),
        "gsig_bf": bf16(gen_sigma_emb),
        "wg_aug": np.ascontiguousarray(np.concatenate(
            [W_gen, b_gen.reshape(N, 1), np.zeros((N, 1), np.float32)],
            axis=1).astype(ml_dtypes.bfloat16)),
        "wg8": np.ascontiguousarray(
            (W_gen.T * WSCALE).astype(ml_dtypes.float8_e4m3)),
        "ident": np.eye(128, dtype=np.float32),
        "ident_bf": np.eye(128, dtype=ml_dtypes.bfloat16),
    }
    if with_bgen:
        NCH = (N + CHUNK - 1) // CHUNK
        bg = np.zeros((NCH * CHUNK,), dtype=ml_dtypes.bfloat16)
        bg[:N] = b_gen.astype(ml_dtypes.bfloat16)
        shared["bgen2d"] = bg.reshape(NCH, CHUNK)

    in_maps = []
    for s in range(NCORES):
        lo, hi = s * BS, (s + 1) * BS
        csh = ctx[lo:hi]                      # [BS, C]
        m = dict(shared)
        m["ctx_idx"] = np.ascontiguousarray(
            csh.reshape(BS * C).reshape(NT, 128).T)
        m["ctx_byc"] = np.ascontiguousarray(
            np.concatenate([csh[k * 128:(k + 1) * 128, :] for k in range(M)],
                           axis=1))
        m["x_idx"] = np.ascontiguousarray(x_batch[lo:hi].reshape(M, 128).T)
        m["epsT"] = np.ascontiguousarray(eps[lo:hi].T)
        in_maps.append(m)
    return in_maps


def kernel(x_batch, context_words_batch, eps, inf_emb, W_aff, b_aff,
           W_mu, b_mu, W_sig, b_sig, gen_sigma_emb, W_gen, b_gen,
           trace=False):
    with_bgen = bool(np.any(np.asarray(b_gen) != 0))
    if with_bgen not in _CACHE:
        _CACHE[with_bgen] = _build(with_bgen)
    nc = _CACHE[with_bgen]

    in_maps = _prep_inputs(x_batch, context_words_batch, eps, inf_emb, W_aff,
                           b_aff, W_mu, b_mu, W_sig, b_sig, gen_sigma_emb,
                           W_gen, b_gen, with_bgen)
    res = run_bass_kernel_spmd(nc, in_maps, core_ids=list(range(NCORES)),
                               trace=trace)
    parts = [res.results[s]["loss_part"] for s in range(NCORES)]
    loss = np.concatenate(parts).astype(np.float64).mean()
    out = np.float32(loss)
    if trace:
        kernel.last_results = res
    return out
